# revision 1
# baseline (speedup 1.0000x reference)
"""CrossAttention Trainium2 kernel — 8-core tensor-parallel (2 heads/core).

Self-contained: builds a Bass/Tile kernel, shards the full inputs across the
8 NeuronCores, runs via the axon PJRT path, and gathers the full output.
"""

import sys
import time

for _p in ("/opt/trn_rl_repo", "/root/.axon_site/_ro/trn_rl_repo"):
    if _p not in sys.path:
        sys.path.insert(0, _p)

import numpy as np
from contextlib import ExitStack

import concourse.bacc as bacc
import concourse.mybir as mybir
import concourse.tile as tile
from concourse.mybir import ActivationFunctionType as AF
from concourse.mybir import AluOpType as ALU

# ---------------------------------------------------------------- problem dims
D = 1024
H = 16
DH = 64
TQ = 2048
TKV = 2048
B = 2
NCORES = 8
HPC = H // NCORES          # heads per core = 2
MPC = HPC * DH             # dims per core  = 128
T = B * TQ                 # token axis (b-grouped) = 4096
NROT = 32                  # rotated channels per head (frac 0.5 of 64)
NHEADS_ROT = 12            # rotated heads (frac 0.75 of 16)
MAX_WL = 8192.0

F32 = mybir.dt.float32

TWO_PI = 2.0 * np.pi
INV_2PI = float(np.float32(1.0 / TWO_PI))
MAGIC = float(np.float32(1.5 * 2 ** 23))
CW1 = float(np.float32(6.28125))
CW2 = float(np.float32(TWO_PI - 6.28125))
CW3 = float(TWO_PI - CW1 - float(np.float32(TWO_PI - 6.28125)))


# ---------------------------------------------------------------- bass builder
def build_kernel(use_mask: bool, debug: bool = False):
    nc = bacc.Bacc("TRN2", target_bir_lowering=False, debug=False,
                   enable_asserts=True, num_devices=NCORES)

    xqT = nc.dram_tensor("xqT", [D, T], F32, kind="ExternalInput").ap()
    xkvT = nc.dram_tensor("xkvT", [D, T], F32, kind="ExternalInput").ap()
    wqT = nc.dram_tensor("wqT", [D, MPC], F32, kind="ExternalInput").ap()
    wkT = nc.dram_tensor("wkT", [D, MPC], F32, kind="ExternalInput").ap()
    wvT = nc.dram_tensor("wvT", [D, MPC], F32, kind="ExternalInput").ap()
    wqTs = nc.dram_tensor("wqTs", [D, MPC], F32, kind="ExternalInput").ap()
    wkTs = nc.dram_tensor("wkTs", [D, MPC], F32, kind="ExternalInput").ap()
    bq_d = nc.dram_tensor("bq", [MPC, 1], F32, kind="ExternalInput").ap()
    bk_d = nc.dram_tensor("bk", [MPC, 1], F32, kind="ExternalInput").ap()
    bv_d = nc.dram_tensor("bv", [1, MPC], F32, kind="ExternalInput").ap()
    bqs_d = nc.dram_tensor("bqs", [MPC, 1], F32, kind="ExternalInput").ap()
    bks_d = nc.dram_tensor("bks", [MPC, 1], F32, kind="ExternalInput").ap()
    wo0_d = nc.dram_tensor("woT0", [128, D], F32, kind="ExternalInput").ap()
    wo1_d = nc.dram_tensor("woT1", [128, D], F32, kind="ExternalInput").ap()
    invf_d = nc.dram_tensor("invf", [1, 128], F32, kind="ExternalInput").ap()
    posq_d = nc.dram_tensor("posq", [1, T], F32, kind="ExternalInput").ap()
    posk_d = nc.dram_tensor("posk", [1, T], F32, kind="ExternalInput").ap()
    if use_mask:
        mask_d = nc.dram_tensor("maskT", [TKV, B, TQ], F32, kind="ExternalInput").ap()
    outT = nc.dram_tensor("outT", [D, T], F32, kind="ExternalOutput").ap()
    dbg = {}
    if debug:
        for nm, shp in (("d_qT", [MPC, T]), ("d_kT", [MPC, T]),
                        ("d_shq", [MPC, T]), ("d_shk", [MPC, T]),
                        ("d_vn0", [128, B * (TKV // 128) * 128]),
                        ("d_vn1", [128, B * (TKV // 128) * 128]),
                        ("d_ot0", [128, T]), ("d_ot1", [128, T]),
                        ("d_pt", [128, 512]), ("d_rb", [128, 512]),
                        ("d_sums", [1, 512]),
                        ("d_st", [128, 512]), ("d_sin", [128, 512]),
                        ("d_cos", [128, 512])):
            dbg[nm] = nc.dram_tensor(nm, shp, F32, kind="ExternalOutput").ap()

    KC = D // 128            # 8 contraction chunks for projections
    TB = T // 512            # 8 token blocks of 512
    QB = TQ // 512           # 4 query blocks per batch
    KVC = TKV // 128         # 16 kv chunks per batch
    VW = 128                 # v chunk width: [ones, zeros, dims]

    with tile.TileContext(nc) as tc:
        with ExitStack() as octx:
            persist = octx.enter_context(tc.tile_pool(name="persist", bufs=1))

            qT = persist.tile([MPC, T], F32, tag="qT")
            kT = persist.tile([MPC, T], F32, tag="kT")
            vn = [persist.tile([128, B * KVC * VW], F32, tag=f"vn{h}",
                                 name=f"vn{h}") for h in range(HPC)]
            vn3 = [v.rearrange("p (c w) -> p c w", w=VW) for v in vn]
            ot0 = persist.tile([128, T], F32, tag="ot0")
            ot1 = persist.tile([128, T], F32, tag="ot1")
            wo0 = persist.tile([128, D], F32, tag="wo0")
            wo1 = persist.tile([128, D], F32, tag="wo1")
            invf_sb = persist.tile([1, 128], F32, tag="invf")
            bq_sb = persist.tile([MPC, 1], F32, tag="bq")
            bk_sb = persist.tile([MPC, 1], F32, tag="bk")
            bv_sb = persist.tile([1, MPC], F32, tag="bv")
            bqs_sb = persist.tile([MPC, 1], F32, tag="bqs")
            bks_sb = persist.tile([MPC, 1], F32, tag="bks")
            ones_row = persist.tile([1, 128], F32, tag="ones_row")

            nc.sync.dma_start(wo0[:], wo0_d[:])
            nc.sync.dma_start(wo1[:], wo1_d[:])
            nc.sync.dma_start(invf_sb[:], invf_d[:])
            nc.sync.dma_start(bq_sb[:], bq_d[:])
            nc.sync.dma_start(bk_sb[:], bk_d[:])
            nc.sync.dma_start(bv_sb[:], bv_d[:])
            nc.sync.dma_start(bqs_sb[:], bqs_d[:])
            nc.sync.dma_start(bks_sb[:], bks_d[:])
            nc.vector.memset(ones_row[:], 1.0)
            nc.vector.memset(ot0[0:64, :], 0.0)
            nc.vector.memset(ot0[0:1, :], 1.0)            # ones row for bo
            nc.vector.memset(ot1[0:64, :], 0.0)
            for h in range(HPC):
                nc.vector.memset(vn3[h][:, :, 0:1], 1.0)  # ones cols for sums
                nc.vector.memset(vn3[h][:, :, 1:DH], 0.0)

            wq_sb, wk_sb, wv_sb, wqs_sb, wks_sb = [], [], [], [], []
            for kc in range(KC):
                for lst, src, tg in ((wq_sb, wqT, "wq"), (wk_sb, wkT, "wk"),
                                     (wv_sb, wvT, "wv"), (wqs_sb, wqTs, "wqs"),
                                     (wks_sb, wkTs, "wks")):
                    t = persist.tile([128, MPC], F32, tag=f"{tg}{kc}",
                                     name=f"{tg}{kc}")
                    nc.sync.dma_start(t[:], src[kc * 128:(kc + 1) * 128, :])
                    lst.append(t)

            # ---------------- phases 1+1.5 share the swapped projections ---
            midctx = ExitStack()
            mid = midctx.enter_context(tc.tile_pool(name="mid", bufs=1))
            shq = mid.tile([MPC, T], F32, tag="shq")
            shk = mid.tile([MPC, T], F32, tag="shk")

            # ---------------- phase 1: q/k/v projections -------------------
            with ExitStack() as ctx:
                xpool = ctx.enter_context(tc.tile_pool(name="xio", bufs=8))
                pjq = ctx.enter_context(tc.tile_pool(name="pjq", bufs=2, space="PSUM"))
                pjk = ctx.enter_context(tc.tile_pool(name="pjk", bufs=2, space="PSUM"))
                pjv = ctx.enter_context(tc.tile_pool(name="pjv", bufs=1, space="PSUM"))
                pjqs = ctx.enter_context(tc.tile_pool(name="pjqs", bufs=1, space="PSUM"))
                pjks = ctx.enter_context(tc.tile_pool(name="pjks", bufs=1, space="PSUM"))

                for tb in range(TB):
                    q_ps = pjq.tile([MPC, 512], F32, tag="q_ps")
                    k_ps = pjk.tile([MPC, 512], F32, tag="k_ps")
                    v_ps = pjv.tile([128, 512], F32, tag="v_ps")
                    qs_ps = pjqs.tile([MPC, 512], F32, tag="qs_ps")
                    ks_ps = pjks.tile([MPC, 512], F32, tag="ks_ps")
                    xq_ts, xkv_ts = [], []
                    for kc in range(KC):
                        xq_t = xpool.tile([128, 512], F32, tag="xq")
                        nc.sync.dma_start(
                            xq_t[:], xqT[kc * 128:(kc + 1) * 128,
                                         tb * 512:(tb + 1) * 512])
                        xkv_t = xpool.tile([128, 512], F32, tag="xkv")
                        nc.sync.dma_start(
                            xkv_t[:], xkvT[kc * 128:(kc + 1) * 128,
                                           tb * 512:(tb + 1) * 512])
                        xq_ts.append(xq_t)
                        xkv_ts.append(xkv_t)
                        st = kc == 0
                        sp = kc == KC - 1
                        nc.tensor.matmul(q_ps[:], wq_sb[kc][:], xq_ts[kc][:],
                                         start=st, stop=sp)
                        nc.tensor.matmul(k_ps[:], wk_sb[kc][:], xkv_ts[kc][:],
                                         start=st, stop=sp)
                        nc.tensor.matmul(qs_ps[:], wqs_sb[kc][:], xq_ts[kc][:],
                                         start=st, stop=sp)
                        nc.tensor.matmul(ks_ps[:], wks_sb[kc][:], xkv_ts[kc][:],
                                         start=st, stop=sp)
                    # v natural layout: one sequential accumulation group per
                    # 128-token column slice (interleaved groups in one PSUM
                    # bank are rejected); bias added via K=1 ones x bv matmul.
                    for tc4 in range(4):
                        for kc in range(KC):
                            nc.tensor.matmul(
                                v_ps[:, tc4 * 128:(tc4 + 1) * 128],
                                xkv_ts[kc][:, tc4 * 128:(tc4 + 1) * 128],
                                wv_sb[kc][:], start=(kc == 0), stop=False)
                        nc.tensor.matmul(v_ps[:, tc4 * 128:(tc4 + 1) * 128],
                                         ones_row[:], bv_sb[:],
                                         start=False, stop=True)
                    nc.scalar.activation(qT[:, tb * 512:(tb + 1) * 512], q_ps[:],
                                         AF.Identity, bias=bq_sb[:])
                    nc.scalar.activation(kT[:, tb * 512:(tb + 1) * 512], k_ps[:],
                                         AF.Identity, bias=bk_sb[:])
                    nc.scalar.activation(shq[:, tb * 512:(tb + 1) * 512],
                                         qs_ps[:], AF.Identity, bias=bqs_sb[:])
                    nc.scalar.activation(shk[:, tb * 512:(tb + 1) * 512],
                                         ks_ps[:], AF.Identity, bias=bks_sb[:])
                    # scatter v chunks into per-head stationary tensors
                    glob = tb * 4
                    for tc4 in range(4):
                        ch = glob + tc4          # global 128-token chunk index
                        for h in range(HPC):
                            nc.scalar.activation(
                                vn[h][:, ch * VW + DH:ch * VW + 2 * DH],
                                v_ps[:, tc4 * 128 + h * DH:
                                     tc4 * 128 + (h + 1) * DH],
                                AF.Identity)

            # ---------------- phase 1.5: rotary on qT / kT -----------------
            # Tables are full-height [128, 512] with rows aligned to the x
            # rows they rotate (rows 0-31 head0, 64-95 head1; rest unused) so
            # every DVE op sees partition-aligned operands.
            with ExitStack() as ctx:
                rot = ctx.enter_context(tc.tile_pool(name="rot", bufs=2))
                radp = ctx.enter_context(tc.tile_pool(name="radp", bufs=2,
                                                      space="PSUM"))
                for src, xt, shx in ((0, qT, shq), (1, kT, shk)):
                    pos_d = posq_d if src == 0 else posk_d
                    for blk in range(TB):
                        sl = slice(blk * 512, (blk + 1) * 512)
                        pos_t = rot.tile([1, 512], F32, tag="pos")
                        nc.sync.dma_start(pos_t[:], pos_d[:, sl])
                        rad = radp.tile([128, 512], F32, tag="rad")
                        nc.tensor.matmul(rad[:], invf_sb[:], pos_t[:],
                                         start=True, stop=True)
                        kr = rot.tile([128, 512], F32, tag="kr")
                        nc.vector.tensor_scalar(kr[:], rad[:], INV_2PI, MAGIC,
                                                ALU.mult, ALU.add)
                        nc.vector.tensor_scalar_sub(kr[:], kr[:], MAGIC)
                        radm = rot.tile([128, 512], F32, tag="radm")
                        nc.vector.cody_waite_cascade(radm[:], rad[:], kr[:],
                                                     CW1, CW2, CW3)
                        wrap0 = rot.tile([128, 512], F32, tag="wrap0")
                        nc.vector.add_range_wrap(wrap0[:], radm[:], 0.0,
                                                 float(np.pi), float(TWO_PI))
                        sin_t = rot.tile([128, 512], F32, tag="sin")
                        nc.scalar.activation(sin_t[:], wrap0[:], AF.Sin)
                        wrap = rot.tile([128, 512], F32, tag="wrap")
                        nc.vector.add_range_wrap(wrap[:], radm[:],
                                                 float(np.pi / 2),
                                                 float(np.pi), float(TWO_PI))
                        cos_t = rot.tile([128, 512], F32, tag="cos")
                        nc.scalar.activation(cos_t[:], wrap[:], AF.Sin)
                        if debug and src == 0 and blk == 0:
                            nc.sync.dma_start(dbg["d_sin"][:], sin_t[:])
                            nc.sync.dma_start(dbg["d_cos"][:], cos_t[:])

                        t1 = rot.tile([128, 512], F32, tag="t1")
                        t2 = rot.tile([128, 512], F32, tag="t2")
                        for h in range(HPC):
                            r = slice(h * DH, h * DH + NROT)
                            nc.vector.tensor_mul(t1[r, :], xt[r, sl],
                                                 cos_t[r, :])
                            nc.vector.tensor_mul(t2[r, :], shx[r, sl],
                                                 sin_t[r, :])
                            nc.vector.tensor_add(xt[r, sl], t1[r, :],
                                                 t2[r, :])

            if debug:
                nc.sync.dma_start(dbg["d_shq"][:], shq[:])
                nc.sync.dma_start(dbg["d_shk"][:], shk[:])
            midctx.close()

            # ---------------- phase 2: attention per (b, qb, h) ------------
            with ExitStack() as ctx:
                stp = ctx.enter_context(tc.tile_pool(name="stp", bufs=4,
                                                     space="PSUM"))
                otp = ctx.enter_context(tc.tile_pool(name="otp", bufs=2,
                                                     space="PSUM"))
                ptp = ctx.enter_context(tc.tile_pool(name="ptp", bufs=18))
                sml = ctx.enter_context(tc.tile_pool(name="sml", bufs=2))
                if use_mask:
                    mkp = ctx.enter_context(tc.tile_pool(name="mkp", bufs=17))

                for b in range(B):
                    for qb in range(QB):
                        qsl = slice(b * TQ + qb * 512, b * TQ + (qb + 1) * 512)
                        mtiles = []
                        if use_mask:
                            for kc in range(KVC):
                                mt = mkp.tile([128, 512], F32, tag="mk")
                                nc.sync.dma_start(
                                    mt[:], mask_d[kc * 128:(kc + 1) * 128, b,
                                                  qb * 512:(qb + 1) * 512])
                                mtiles.append(mt)
                        for h in range(HPC):
                            hs = slice(h * DH, (h + 1) * DH)
                            pts = []
                            for kc in range(KVC):
                                st_ps = stp.tile([128, 512], F32, tag="st")
                                nc.tensor.matmul(
                                    st_ps[:],
                                    kT[hs, b * TKV + kc * 128:
                                       b * TKV + (kc + 1) * 128],
                                    qT[hs, qsl], start=True, stop=True)
                                pt = ptp.tile([128, 512], F32, tag="pt")
                                if debug and b == 0 and qb == 0 and h == 0 and kc == 0:
                                    st_sb = ptp.tile([128, 512], F32, tag="st_sb", bufs=1)
                                    nc.vector.tensor_copy(st_sb[:], st_ps[:])
                                    nc.sync.dma_start(dbg["d_st"][:], st_sb[:])
                                nc.scalar.activation(pt[:], st_ps[:], AF.Exp)
                                if use_mask:
                                    nc.vector.tensor_mul(pt[:], pt[:],
                                                          mtiles[kc][:])
                                if debug and b == 0 and qb == 0 and h == 0 and kc == 0:
                                    nc.sync.dma_start(dbg["d_pt"][:], pt[:])
                                pts.append(pt)
                            ot_ps = otp.tile([128, 512], F32, tag="ot")
                            for kc in range(KVC):
                                cw = (b * KVC + kc) * VW
                                nc.tensor.matmul(ot_ps[:],
                                                 vn[h][:, cw:cw + VW],
                                                 pts[kc][:],
                                                 start=(kc == 0),
                                                 stop=(kc == KVC - 1))
                            recip = sml.tile([1, 512], F32, tag="recip")
                            nc.vector.reciprocal_approx_fast(
                                out=recip[:], in_=ot_ps[0:1, :])
                            rb = sml.tile([128, 512], F32, tag="rb")
                            nc.gpsimd.partition_broadcast(rb[:], recip[:])
                            if debug and b == 0 and qb == 0 and h == 0:
                                nc.sync.dma_start(dbg["d_rb"][:], rb[:])
                                nc.sync.dma_start(dbg["d_sums"][:],
                                                  recip[:])
                            dst = ot0 if h == 0 else ot1
                            nc.vector.tensor_mul(dst[DH:2 * DH, qsl],
                                                 ot_ps[DH:2 * DH, :],
                                                 rb[DH:2 * DH, :])

            if debug:
                nc.sync.dma_start(dbg["d_qT"][:], qT[:])
                nc.sync.dma_start(dbg["d_kT"][:], kT[:])
                nc.sync.dma_start(dbg["d_vn0"][:], vn[0][:])
                nc.sync.dma_start(dbg["d_vn1"][:], vn[1][:])
                nc.sync.dma_start(dbg["d_ot0"][:], ot0[:])
                nc.sync.dma_start(dbg["d_ot1"][:], ot1[:])

            # ---------------- phase 3: output projection -------------------
            with ExitStack() as ctx:
                outp = ctx.enter_context(tc.tile_pool(name="outp", bufs=4,
                                                      space="PSUM"))
                osb = ctx.enter_context(tc.tile_pool(name="osb", bufs=4))
                for jc in range(KC):
                    for tb in range(TB):
                        o_ps = outp.tile([128, 512], F32, tag="o")
                        nc.tensor.matmul(o_ps[:],
                                         wo0[:, jc * 128:(jc + 1) * 128],
                                         ot0[:, tb * 512:(tb + 1) * 512],
                                         start=True, stop=False)
                        nc.tensor.matmul(o_ps[:],
                                         wo1[:, jc * 128:(jc + 1) * 128],
                                         ot1[:, tb * 512:(tb + 1) * 512],
                                         start=False, stop=True)
                        o_sb = osb.tile([128, 512], F32, tag="o_sb")
                        nc.scalar.activation(o_sb[:], o_ps[:], AF.Identity)
                        nc.sync.dma_start(
                            outT[jc * 128:(jc + 1) * 128,
                                 tb * 512:(tb + 1) * 512], o_sb[:])

    nc.compile()
    return nc


# ---------------------------------------------------------------- pjrt runner
def _make_runner(nc, n_cores=NCORES):
    import jax
    from jax.sharding import Mesh, PartitionSpec
    from jax.experimental.shard_map import shard_map
    from concourse.bass2jax import (_bass_exec_p, install_neuronx_cc_hook,
                                    partition_id_tensor)

    install_neuronx_cc_hook()
    partition_name = (nc.partition_id_tensor.name
                      if nc.partition_id_tensor else None)
    in_names, out_names, out_avals, zero_shapes = [], [], [], []
    for alloc in nc.m.functions[0].allocations:
        if not isinstance(alloc, mybir.MemoryLocationSet):
            continue
        name = alloc.memorylocations[0].name
        if alloc.kind == "ExternalInput":
            if name != partition_name:
                in_names.append(name)
        elif alloc.kind == "ExternalOutput":
            shape = tuple(alloc.tensor_shape)
            dtype = mybir.dt.np(alloc.dtype)
            out_names.append(name)
            out_avals.append(jax.core.ShapedArray(shape, dtype))
            zero_shapes.append((shape, dtype))
    n_params = len(in_names)
    n_outs = len(out_avals)
    all_in_names = list(in_names) + list(out_names)
    if partition_name is not None:
        all_in_names.append(partition_name)

    def _body(*args):
        operands = list(args)
        if partition_name is not None:
            operands.append(partition_id_tensor())
        return tuple(_bass_exec_p.bind(
            *operands, out_avals=tuple(out_avals), in_names=tuple(all_in_names),
            out_names=tuple(out_names), lowering_input_output_aliases=(),
            sim_require_finite=True, sim_require_nnan=True, nc=nc))

    devices = jax.devices()[:n_cores]
    mesh = Mesh(np.asarray(devices), ("core",))
    in_specs = (PartitionSpec("core"),) * (n_params + n_outs)
    out_specs = (PartitionSpec("core"),) * len(out_names)
    donate = tuple(range(n_params, n_params + n_outs))
    sharded = jax.jit(
        shard_map(_body, mesh=mesh, in_specs=in_specs, out_specs=out_specs,
                  check_rep=False),
        donate_argnums=donate, keep_unused=True)

    def run(in_maps, time_iters=0):
        per_core = [[np.asarray(m[name]) for name in in_names]
                    for m in in_maps]
        concat_in = [np.concatenate([per_core[c][i] for c in range(n_cores)],
                                    axis=0) for i in range(n_params)]

        def zeros():
            return [np.zeros((n_cores * s[0], *s[1:]), d)
                    for s, d in zero_shapes]

        import jax
        out_arrs = sharded(*concat_in, *zeros())
        jax.block_until_ready(out_arrs)
        times = []
        for _ in range(time_iters):
            t0 = time.perf_counter()
            o = sharded(*concat_in, *zeros())
            jax.block_until_ready(o)
            times.append(time.perf_counter() - t0)
            out_arrs = o
        results = [
            {name: np.asarray(out_arrs[i]).reshape(n_cores,
                                                   *out_avals[i].shape)[c]
             for i, name in enumerate(out_names)}
            for c in range(n_cores)]
        return results, times

    return run


# ---------------------------------------------------------------- host shard
def _inv_freq_signed():
    nb = NROT // 2  # 16 distinct frequencies
    freq = MAX_WL ** (2.0 / NROT * np.linspace(0.0, float(nb), nb))
    inv = (1.0 / freq).astype(np.float32)
    s = np.zeros(128, np.float32)
    for h in range(HPC):
        s[h * DH:h * DH + NROT:2] = -inv
        s[h * DH + 1:h * DH + NROT:2] = inv
    return s


def make_in_maps(inputs_q, inputs_kv, mask, q_positions, kv_positions,
                 Wq, bq, Wk, bk, Wv, bv, Wo, bo, use_mask):
    f32 = np.float32
    xqT = np.ascontiguousarray(
        np.asarray(inputs_q, f32).transpose(2, 1, 0).reshape(D, T))
    xkvT = np.ascontiguousarray(
        np.asarray(inputs_kv, f32).transpose(2, 1, 0).reshape(D, T))
    posq = np.ascontiguousarray(
        np.asarray(q_positions, f32).T.reshape(1, T))
    posk = np.ascontiguousarray(
        np.asarray(kv_positions, f32).T.reshape(1, T))
    scale = f32(1.0 / np.sqrt(DH))
    perm = np.arange(MPC)
    for h in range(HPC):
        base = h * DH
        perm[base:base + NROT:2] = np.arange(base + 1, base + NROT + 1, 2)
        perm[base + 1:base + NROT:2] = np.arange(base, base + NROT, 2)
    Wq, Wk, Wv, Wo = (np.asarray(a, f32) for a in (Wq, Wk, Wv, Wo))
    bq, bk, bv, bo = (np.asarray(a, f32) for a in (bq, bk, bv, bo))
    invf = _inv_freq_signed()
    if use_mask:
        maskT = np.ascontiguousarray((np.asarray(mask) > 0).astype(f32))

    in_maps = []
    for c in range(NCORES):
        sl = slice(c * MPC, (c + 1) * MPC)
        m = {
            "xqT": xqT, "xkvT": xkvT, "posq": posq, "posk": posk,
            "wqT": np.ascontiguousarray((scale * Wq[sl, :]).T),
            "wkT": np.ascontiguousarray(Wk[sl, :].T),
            "wvT": np.ascontiguousarray(Wv[sl, :].T),
            "wqTs": np.ascontiguousarray((scale * Wq[sl, :][perm, :]).T),
            "wkTs": np.ascontiguousarray(Wk[sl, :][perm, :].T),
            "bq": (scale * bq[sl]).reshape(MPC, 1),
            "bk": bk[sl].reshape(MPC, 1).copy(),
            "bv": bv[sl].reshape(1, MPC).copy(),
            "bqs": (scale * bq[sl])[perm].reshape(MPC, 1).copy(),
            "bks": bk[sl][perm].reshape(MPC, 1).copy(),
            "woT0": np.ascontiguousarray(np.concatenate(
                [(bo if c == 0 else np.zeros_like(bo))[None, :],
                 np.zeros((DH - 1, D), f32),
                 Wo[:, c * MPC:c * MPC + DH].T], axis=0)),
            "woT1": np.ascontiguousarray(np.concatenate(
                [np.zeros((DH, D), f32),
                 Wo[:, c * MPC + DH:(c + 1) * MPC].T], axis=0)),
            "invf": (invf if (c + 1) * HPC <= NHEADS_ROT
                     else np.zeros_like(invf)).reshape(1, 128),
        }
        if use_mask:
            m["maskT"] = maskT
        in_maps.append(m)
    return in_maps


_CACHE = {}


def _get(use_mask):
    if use_mask not in _CACHE:
        nc = build_kernel(use_mask)
        _CACHE[use_mask] = (nc, _make_runner(nc))
    return _CACHE[use_mask]


def kernel(inputs_q, inputs_kv, mask, q_positions, kv_positions,
           Wq, bq, Wk, bk, Wv, bv, Wo, bo, _time_iters=0):
    use_mask = not bool(np.all(np.asarray(mask) > 0))
    nc, run = _get(use_mask)
    in_maps = make_in_maps(inputs_q, inputs_kv, mask, q_positions,
                           kv_positions, Wq, bq, Wk, bk, Wv, bv, Wo, bo,
                           use_mask)
    results, times = run(in_maps, time_iters=_time_iters)
    acc = np.zeros((D, T), np.float64)
    for c in range(NCORES):
        acc += results[c]["outT"]
    out = acc.astype(np.float32).reshape(D, B, TQ).transpose(2, 1, 0)
    out = np.ascontiguousarray(out)
    if _time_iters:
        kernel._last_times = times
    return out



# revision 32
# speedup vs baseline: 2.2127x; 2.2127x over previous
"""CrossAttention Trainium2 kernel — 8-core tensor-parallel (2 heads/core).

Self-contained: builds a Bass/Tile kernel, shards the full inputs across the
8 NeuronCores, runs via the axon PJRT path, and gathers the full output.
"""

import sys
import time

for _p in ("/opt/trn_rl_repo", "/root/.axon_site/_ro/trn_rl_repo"):
    if _p not in sys.path:
        sys.path.insert(0, _p)

import numpy as np
from contextlib import ExitStack

import concourse.bacc as bacc
import concourse.mybir as mybir
import concourse.tile as tile
from concourse.mybir import ActivationFunctionType as AF
from concourse.mybir import AluOpType as ALU

# ---------------------------------------------------------------- problem dims
D = 1024
H = 16
DH = 64
TQ = 2048
TKV = 2048
B = 2
NCORES = 8
HPC = H // NCORES          # heads per core = 2
MPC = HPC * DH             # dims per core  = 128
T = B * TQ                 # token axis (b-grouped) = 4096
NROT = 32                  # rotated channels per head (frac 0.5 of 64)
NHEADS_ROT = 12            # rotated heads (frac 0.75 of 16)
MAX_WL = 8192.0

F32 = mybir.dt.float32
F16 = mybir.dt.float16
NPF16 = np.float16

TWO_PI = 2.0 * np.pi
INV_2PI = float(np.float32(1.0 / TWO_PI))
MAGIC = float(np.float32(1.5 * 2 ** 23))
CW1 = float(np.float32(6.28125))
CW2 = float(np.float32(TWO_PI - 6.28125))
CW3 = float(TWO_PI - CW1 - float(np.float32(TWO_PI - 6.28125)))


# ---------------------------------------------------------------- bass builder
def build_kernel(use_mask: bool, debug: bool = False):
    nc = bacc.Bacc("TRN2", target_bir_lowering=False, debug=False,
                   enable_asserts=True, num_devices=NCORES)

    xqT = nc.dram_tensor("xqT", [D, T], F16, kind="ExternalInput").ap()
    xkvT = nc.dram_tensor("xkvT", [D, T], F16, kind="ExternalInput").ap()
    wqT = nc.dram_tensor("wqT", [D, MPC], F16, kind="ExternalInput").ap()
    wkT = nc.dram_tensor("wkT", [D, MPC], F16, kind="ExternalInput").ap()
    wvT = nc.dram_tensor("wvT", [D, MPC], F16, kind="ExternalInput").ap()
    wqTs = nc.dram_tensor("wqTs", [D, MPC], F16, kind="ExternalInput").ap()
    wkTs = nc.dram_tensor("wkTs", [D, MPC], F16, kind="ExternalInput").ap()
    bq_d = nc.dram_tensor("bq", [MPC, 1], F32, kind="ExternalInput").ap()
    bk_d = nc.dram_tensor("bk", [MPC, 1], F32, kind="ExternalInput").ap()
    bv_d = nc.dram_tensor("bv", [1, MPC], F16, kind="ExternalInput").ap()
    bqs_d = nc.dram_tensor("bqs", [MPC, 1], F32, kind="ExternalInput").ap()
    bks_d = nc.dram_tensor("bks", [MPC, 1], F32, kind="ExternalInput").ap()
    wo0_d = nc.dram_tensor("woT0", [128, D], F16, kind="ExternalInput").ap()
    wo1_d = nc.dram_tensor("woT1", [128, D], F16, kind="ExternalInput").ap()
    invf_d = nc.dram_tensor("invf", [1, 128], F32, kind="ExternalInput").ap()
    posq_d = nc.dram_tensor("posq", [1, T], F32, kind="ExternalInput").ap()
    posk_d = nc.dram_tensor("posk", [1, T], F32, kind="ExternalInput").ap()
    if use_mask:
        mask_d = nc.dram_tensor("maskT", [TKV, B, TQ], F16, kind="ExternalInput").ap()
    outT = nc.dram_tensor("outT", [D, T], F16, kind="ExternalOutput").ap()
    dbg = {}
    if debug:
        for nm, shp in (("d_qT", [MPC, T]), ("d_kT", [MPC, T]),
                        ("d_shq", [MPC, T]), ("d_shk", [MPC, T]),
                        ("d_vn0", [128, B * (TKV // 128) * 128]),
                        ("d_vn1", [128, B * (TKV // 128) * 128]),
                        ("d_ot0", [128, T]), ("d_ot1", [128, T]),
                        ("d_pt", [128, 512]), ("d_rb", [128, 512]),
                        ("d_sums", [1, 512]),
                        ("d_st", [128, 512]), ("d_sin", [128, 512]),
                        ("d_cos", [128, 512])):
            dbg[nm] = nc.dram_tensor(nm, shp, F32, kind="ExternalOutput").ap()

    KC = D // 128            # 8 contraction chunks for projections
    TB = T // 512            # 8 token blocks of 512
    QB = TQ // 512           # 4 query blocks per batch
    KVC = TKV // 128         # 16 kv chunks per batch
    VW = 128                 # v chunk width: [ones, zeros, dims]

    with tile.TileContext(nc) as tc:
        with ExitStack() as octx:
            persist = octx.enter_context(tc.tile_pool(name="persist", bufs=1))

            qT = persist.tile([MPC, T], F16, tag="qT")
            kT = persist.tile([MPC, T], F16, tag="kT")
            vn = [persist.tile([128, B * KVC * VW], F16, tag=f"vn{h}",
                                 name=f"vn{h}") for h in range(HPC)]
            vn3 = [v.rearrange("p (c w) -> p c w", w=VW) for v in vn]
            ot0 = persist.tile([128, T], F16, tag="ot0")
            ot1 = persist.tile([128, T], F16, tag="ot1")
            wo0 = persist.tile([128, D], F16, tag="wo0")
            wo1 = persist.tile([128, D], F16, tag="wo1")
            invf_sb = persist.tile([1, 128], F32, tag="invf")
            bq_sb = persist.tile([MPC, 1], F32, tag="bq")
            bk_sb = persist.tile([MPC, 1], F32, tag="bk")
            bv_sb = persist.tile([1, MPC], F16, tag="bv")
            bqs_sb = persist.tile([MPC, 1], F32, tag="bqs")
            bks_sb = persist.tile([MPC, 1], F32, tag="bks")
            ones_row = persist.tile([1, 128], F16, tag="ones_row")

            nc.sync.dma_start(wo0[:], wo0_d[:])
            nc.sync.dma_start(wo1[:], wo1_d[:])
            nc.sync.dma_start(invf_sb[:], invf_d[:])
            nc.sync.dma_start(bq_sb[:], bq_d[:])
            nc.sync.dma_start(bk_sb[:], bk_d[:])
            nc.sync.dma_start(bv_sb[:], bv_d[:])
            nc.sync.dma_start(bqs_sb[:], bqs_d[:])
            nc.sync.dma_start(bks_sb[:], bks_d[:])
            nc.vector.memset(ones_row[:], 1.0)
            nc.vector.memset(ot0[0:64, :], 0.0)
            nc.vector.memset(ot0[0:1, :], 1.0)            # ones row for bo
            nc.vector.memset(ot1[0:64, :], 0.0)
            for h in range(HPC):
                nc.vector.memset(vn3[h][:, :, 0:1], 1.0)  # ones cols for sums
                nc.vector.memset(vn3[h][:, :, 1:DH], 0.0)

            wq_sb, wk_sb, wv_sb, wqs_sb, wks_sb = [], [], [], [], []
            for kc in range(KC):
                for lst, src, tg, dtp in (
                        (wq_sb, wqT, "wq", F16), (wk_sb, wkT, "wk", F16),
                        (wv_sb, wvT, "wv", F16), (wqs_sb, wqTs, "wqs", F16),
                        (wks_sb, wkTs, "wks", F16)):
                    t = persist.tile([128, MPC], dtp, tag=f"{tg}{kc}",
                                     name=f"{tg}{kc}")
                    nc.sync.dma_start(t[:], src[kc * 128:(kc + 1) * 128, :])
                    lst.append(t)

            # ---------------- phases 1+1.5 share the swapped projections ---
            midctx = ExitStack()
            mid = midctx.enter_context(tc.tile_pool(name="mid", bufs=1))
            shq = mid.tile([MPC, T], F16, tag="shq")
            shk = mid.tile([MPC, T], F16, tag="shk")

            # ---------------- phase 1: q/k/v projections -------------------
            with ExitStack() as ctx:
                xpool = ctx.enter_context(tc.tile_pool(name="xio", bufs=8))
                pjq = ctx.enter_context(tc.tile_pool(name="pjq", bufs=2, space="PSUM"))
                pjk = ctx.enter_context(tc.tile_pool(name="pjk", bufs=2, space="PSUM"))
                pjv = ctx.enter_context(tc.tile_pool(name="pjv", bufs=1, space="PSUM"))
                pjqs = ctx.enter_context(tc.tile_pool(name="pjqs", bufs=1, space="PSUM"))
                pjks = ctx.enter_context(tc.tile_pool(name="pjks", bufs=1, space="PSUM"))

                for tb in range(TB):
                    q_ps = pjq.tile([MPC, 512], F32, tag="q_ps")
                    k_ps = pjk.tile([MPC, 512], F32, tag="k_ps")
                    v_ps = pjv.tile([128, 512], F32, tag="v_ps")
                    qs_ps = pjqs.tile([MPC, 512], F32, tag="qs_ps")
                    ks_ps = pjks.tile([MPC, 512], F32, tag="ks_ps")
                    xq_ts, xkv_ts = [], []
                    for kc in range(KC):
                        xq_t = xpool.tile([128, 512], F16, tag="xq")
                        nc.sync.dma_start(
                            xq_t[:], xqT[kc * 128:(kc + 1) * 128,
                                         tb * 512:(tb + 1) * 512])
                        xkv_t = xpool.tile([128, 512], F16, tag="xkv")
                        nc.sync.dma_start(
                            xkv_t[:], xkvT[kc * 128:(kc + 1) * 128,
                                           tb * 512:(tb + 1) * 512])
                        xq_ts.append(xq_t)
                        xkv_ts.append(xkv_t)
                        st = kc == 0
                        sp = kc == KC - 1
                        nc.tensor.matmul(q_ps[:], wq_sb[kc][:], xq_ts[kc][:],
                                         start=st, stop=sp)
                        nc.tensor.matmul(k_ps[:], wk_sb[kc][:], xkv_ts[kc][:],
                                         start=st, stop=sp)
                        nc.tensor.matmul(qs_ps[:], wqs_sb[kc][:], xq_ts[kc][:],
                                         start=st, stop=sp)
                        nc.tensor.matmul(ks_ps[:], wks_sb[kc][:], xkv_ts[kc][:],
                                         start=st, stop=sp)
                    # v natural layout: one sequential accumulation group per
                    # 128-token column slice (interleaved groups in one PSUM
                    # bank are rejected); bias added via K=1 ones x bv matmul.
                    for tc4 in range(4):
                        for kc in range(KC):
                            nc.tensor.matmul(
                                v_ps[:, tc4 * 128:(tc4 + 1) * 128],
                                xkv_ts[kc][:, tc4 * 128:(tc4 + 1) * 128],
                                wv_sb[kc][:], start=(kc == 0), stop=False)
                        nc.tensor.matmul(v_ps[:, tc4 * 128:(tc4 + 1) * 128],
                                         ones_row[:], bv_sb[:],
                                         start=False, stop=True)
                    nc.scalar.activation(qT[:, tb * 512:(tb + 1) * 512], q_ps[:],
                                         AF.Identity, bias=bq_sb[:])
                    nc.scalar.activation(kT[:, tb * 512:(tb + 1) * 512], k_ps[:],
                                         AF.Identity, bias=bk_sb[:])
                    nc.scalar.activation(shq[:, tb * 512:(tb + 1) * 512],
                                         qs_ps[:], AF.Identity, bias=bqs_sb[:])
                    nc.scalar.activation(shk[:, tb * 512:(tb + 1) * 512],
                                         ks_ps[:], AF.Identity, bias=bks_sb[:])
                    # scatter v chunks into per-head stationary tensors
                    glob = tb * 4
                    for tc4 in range(4):
                        ch = glob + tc4          # global 128-token chunk index
                        for h in range(HPC):
                            nc.scalar.activation(
                                vn[h][:, ch * VW + DH:ch * VW + 2 * DH],
                                v_ps[:, tc4 * 128 + h * DH:
                                     tc4 * 128 + (h + 1) * DH],
                                AF.Identity)

            # ---------------- phase 1.5: rotary on qT / kT -----------------
            # Tables are full-height [128, 512] with rows aligned to the x
            # rows they rotate (rows 0-31 head0, 64-95 head1; rest unused) so
            # every DVE op sees partition-aligned operands.
            with ExitStack() as ctx:
                rot = ctx.enter_context(tc.tile_pool(name="rot", bufs=2))
                radp = ctx.enter_context(tc.tile_pool(name="radp", bufs=2,
                                                      space="PSUM"))
                for src, xt, shx in ((0, qT, shq), (1, kT, shk)):
                    pos_d = posq_d if src == 0 else posk_d
                    for blk in range(TB):
                        sl = slice(blk * 512, (blk + 1) * 512)
                        pos_t = rot.tile([1, 512], F32, tag="pos")
                        nc.sync.dma_start(pos_t[:], pos_d[:, sl])
                        rad = radp.tile([128, 512], F32, tag="rad")
                        nc.tensor.matmul(rad[:], invf_sb[:], pos_t[:],
                                         start=True, stop=True)
                        kr = rot.tile([128, 512], F32, tag="kr")
                        nc.vector.tensor_scalar(kr[:], rad[:], INV_2PI, MAGIC,
                                                ALU.mult, ALU.add)
                        nc.vector.tensor_scalar_sub(kr[:], kr[:], MAGIC)
                        radm = rot.tile([128, 512], F32, tag="radm")
                        nc.vector.cody_waite_cascade(radm[:], rad[:], kr[:],
                                                     CW1, CW2, CW3)
                        wrap0 = rot.tile([128, 512], F32, tag="wrap0")
                        nc.vector.add_range_wrap(wrap0[:], radm[:], 0.0,
                                                 float(np.pi), float(TWO_PI))
                        sin_t = rot.tile([128, 512], F32, tag="sin")
                        nc.scalar.activation(sin_t[:], wrap0[:], AF.Sin)
                        wrap = rot.tile([128, 512], F32, tag="wrap")
                        nc.vector.add_range_wrap(wrap[:], radm[:],
                                                 float(np.pi / 2),
                                                 float(np.pi), float(TWO_PI))
                        cos_t = rot.tile([128, 512], F32, tag="cos")
                        nc.scalar.activation(cos_t[:], wrap[:], AF.Sin)
                        if debug and src == 0 and blk == 0:
                            nc.sync.dma_start(dbg["d_sin"][:], sin_t[:])
                            nc.sync.dma_start(dbg["d_cos"][:], cos_t[:])

                        t1 = rot.tile([128, 512], F32, tag="t1")
                        t2 = rot.tile([128, 512], F32, tag="t2")
                        for h in range(HPC):
                            r = slice(h * DH, h * DH + NROT)
                            nc.vector.tensor_mul(t1[r, :], xt[r, sl],
                                                 cos_t[r, :])
                            nc.vector.tensor_mul(t2[r, :], shx[r, sl],
                                                 sin_t[r, :])
                            nc.vector.tensor_add(xt[r, sl], t1[r, :],
                                                 t2[r, :])

            if debug:
                nc.sync.dma_start(dbg["d_shq"][:], shq[:])
                nc.sync.dma_start(dbg["d_shk"][:], shk[:])
            midctx.close()

            # ---------------- phase 2: attention per (b, qb, h) ------------
            with ExitStack() as ctx:
                stp = ctx.enter_context(tc.tile_pool(name="stp", bufs=4,
                                                     space="PSUM"))
                otp = ctx.enter_context(tc.tile_pool(name="otp", bufs=2,
                                                     space="PSUM"))
                ptp = ctx.enter_context(tc.tile_pool(name="ptp", bufs=18))
                sml = ctx.enter_context(tc.tile_pool(name="sml", bufs=2))
                if use_mask:
                    mkp = ctx.enter_context(tc.tile_pool(name="mkp", bufs=17))

                for b in range(B):
                    for qb in range(QB):
                        qsl = slice(b * TQ + qb * 512, b * TQ + (qb + 1) * 512)
                        mtiles = []
                        if use_mask:
                            for kc in range(KVC):
                                mt = mkp.tile([128, 512], F32, tag="mk")
                                nc.sync.dma_start(
                                    mt[:], mask_d[kc * 128:(kc + 1) * 128, b,
                                                  qb * 512:(qb + 1) * 512])
                                mtiles.append(mt)
                        for h in range(HPC):
                            hs = slice(h * DH, (h + 1) * DH)
                            pts = []
                            for kc in range(KVC):
                                st_ps = stp.tile([128, 512], F32, tag="st")
                                nc.tensor.matmul(
                                    st_ps[:],
                                    kT[hs, b * TKV + kc * 128:
                                       b * TKV + (kc + 1) * 128],
                                    qT[hs, qsl], start=True, stop=True)
                                pt = ptp.tile([128, 512], F16, tag="pt")
                                if debug and b == 0 and qb == 0 and h == 0 and kc == 0:
                                    st_sb = ptp.tile([128, 512], F32, tag="st_sb", bufs=1)
                                    nc.vector.tensor_copy(st_sb[:], st_ps[:])
                                    nc.sync.dma_start(dbg["d_st"][:], st_sb[:])
                                nc.scalar.activation(pt[:], st_ps[:], AF.Exp)
                                if use_mask:
                                    nc.vector.tensor_mul(pt[:], pt[:],
                                                          mtiles[kc][:])
                                if debug and b == 0 and qb == 0 and h == 0 and kc == 0:
                                    nc.sync.dma_start(dbg["d_pt"][:], pt[:])
                                pts.append(pt)
                            ot_ps = otp.tile([128, 512], F32, tag="ot")
                            for kc in range(KVC):
                                cw = (b * KVC + kc) * VW
                                nc.tensor.matmul(ot_ps[:],
                                                 vn[h][:, cw:cw + VW],
                                                 pts[kc][:],
                                                 start=(kc == 0),
                                                 stop=(kc == KVC - 1))
                            recip = sml.tile([1, 512], F32, tag="recip")
                            nc.vector.reciprocal_approx_fast(
                                out=recip[:], in_=ot_ps[0:1, :])
                            rb = sml.tile([128, 512], F32, tag="rb")
                            nc.gpsimd.partition_broadcast(rb[:], recip[:])
                            if debug and b == 0 and qb == 0 and h == 0:
                                nc.sync.dma_start(dbg["d_rb"][:], rb[:])
                                nc.sync.dma_start(dbg["d_sums"][:],
                                                  recip[:])
                            dst = ot0 if h == 0 else ot1
                            nc.vector.tensor_mul(dst[DH:2 * DH, qsl],
                                                 ot_ps[DH:2 * DH, :],
                                                 rb[DH:2 * DH, :])

            if debug:
                nc.sync.dma_start(dbg["d_qT"][:], qT[:])
                nc.sync.dma_start(dbg["d_kT"][:], kT[:])
                nc.sync.dma_start(dbg["d_vn0"][:], vn[0][:])
                nc.sync.dma_start(dbg["d_vn1"][:], vn[1][:])
                nc.sync.dma_start(dbg["d_ot0"][:], ot0[:])
                nc.sync.dma_start(dbg["d_ot1"][:], ot1[:])

            # ---------------- phase 3: output projection -------------------
            with ExitStack() as ctx:
                outp = ctx.enter_context(tc.tile_pool(name="outp", bufs=4,
                                                      space="PSUM"))
                osb = ctx.enter_context(tc.tile_pool(name="osb", bufs=4))
                for jc in range(KC):
                    for tb in range(TB):
                        o_ps = outp.tile([128, 512], F32, tag="o")
                        nc.tensor.matmul(o_ps[:],
                                         wo0[:, jc * 128:(jc + 1) * 128],
                                         ot0[:, tb * 512:(tb + 1) * 512],
                                         start=True, stop=False)
                        nc.tensor.matmul(o_ps[:],
                                         wo1[:, jc * 128:(jc + 1) * 128],
                                         ot1[:, tb * 512:(tb + 1) * 512],
                                         start=False, stop=True)
                        o_sb = osb.tile([128, 512], F16, tag="o_sb")
                        nc.scalar.activation(o_sb[:], o_ps[:], AF.Identity)
                        nc.sync.dma_start(
                            outT[jc * 128:(jc + 1) * 128,
                                 tb * 512:(tb + 1) * 512], o_sb[:])

    nc.compile()
    return nc


# ---------------------------------------------------------------- pjrt runner
def _make_runner(nc, n_cores=NCORES):
    import jax
    from jax.sharding import Mesh, PartitionSpec
    from jax.experimental.shard_map import shard_map
    from concourse.bass2jax import (_bass_exec_p, install_neuronx_cc_hook,
                                    partition_id_tensor)

    install_neuronx_cc_hook()
    partition_name = (nc.partition_id_tensor.name
                      if nc.partition_id_tensor else None)
    in_names, out_names, out_avals, zero_shapes = [], [], [], []
    for alloc in nc.m.functions[0].allocations:
        if not isinstance(alloc, mybir.MemoryLocationSet):
            continue
        name = alloc.memorylocations[0].name
        if alloc.kind == "ExternalInput":
            if name != partition_name:
                in_names.append(name)
        elif alloc.kind == "ExternalOutput":
            shape = tuple(alloc.tensor_shape)
            dtype = mybir.dt.np(alloc.dtype)
            out_names.append(name)
            out_avals.append(jax.core.ShapedArray(shape, dtype))
            zero_shapes.append((shape, dtype))
    n_params = len(in_names)
    n_outs = len(out_avals)
    all_in_names = list(in_names) + list(out_names)
    if partition_name is not None:
        all_in_names.append(partition_name)

    def _body(*args):
        operands = list(args)
        if partition_name is not None:
            operands.append(partition_id_tensor())
        return tuple(_bass_exec_p.bind(
            *operands, out_avals=tuple(out_avals), in_names=tuple(all_in_names),
            out_names=tuple(out_names), lowering_input_output_aliases=(),
            sim_require_finite=True, sim_require_nnan=True, nc=nc))

    devices = jax.devices()[:n_cores]
    mesh = Mesh(np.asarray(devices), ("core",))
    in_specs = (PartitionSpec("core"),) * (n_params + n_outs)
    out_specs = (PartitionSpec("core"),) * len(out_names)
    donate = tuple(range(n_params, n_params + n_outs))
    sharded = jax.jit(
        shard_map(_body, mesh=mesh, in_specs=in_specs, out_specs=out_specs,
                  check_rep=False),
        donate_argnums=donate, keep_unused=True)

    def run(in_maps, time_iters=0):
        per_core = [[np.asarray(m[name]) for name in in_names]
                    for m in in_maps]
        concat_in = [np.concatenate([per_core[c][i] for c in range(n_cores)],
                                    axis=0) for i in range(n_params)]

        def zeros():
            return [np.zeros((n_cores * s[0], *s[1:]), d)
                    for s, d in zero_shapes]

        import jax
        out_arrs = sharded(*concat_in, *zeros())
        jax.block_until_ready(out_arrs)
        times = []
        for _ in range(time_iters):
            t0 = time.perf_counter()
            o = sharded(*concat_in, *zeros())
            jax.block_until_ready(o)
            times.append(time.perf_counter() - t0)
            out_arrs = o
        results = [
            {name: np.asarray(out_arrs[i]).reshape(n_cores,
                                                   *out_avals[i].shape)[c]
             for i, name in enumerate(out_names)}
            for c in range(n_cores)]
        return results, times

    return run


# ---------------------------------------------------------------- host shard
def _inv_freq_signed():
    nb = NROT // 2  # 16 distinct frequencies
    freq = MAX_WL ** (2.0 / NROT * np.linspace(0.0, float(nb), nb))
    inv = (1.0 / freq).astype(np.float32)
    s = np.zeros(128, np.float32)
    for h in range(HPC):
        s[h * DH:h * DH + NROT:2] = -inv
        s[h * DH + 1:h * DH + NROT:2] = inv
    return s


def make_in_maps(inputs_q, inputs_kv, mask, q_positions, kv_positions,
                 Wq, bq, Wk, bk, Wv, bv, Wo, bo, use_mask):
    f32 = np.float32
    xqT = np.ascontiguousarray(
        np.asarray(inputs_q, f32).transpose(2, 1, 0).reshape(D, T)).astype(NPF16)
    xkvT = np.ascontiguousarray(
        np.asarray(inputs_kv, f32).transpose(2, 1, 0).reshape(D, T)).astype(NPF16)
    posq = np.ascontiguousarray(
        np.asarray(q_positions, f32).T.reshape(1, T))
    posk = np.ascontiguousarray(
        np.asarray(kv_positions, f32).T.reshape(1, T))
    scale = f32(1.0 / np.sqrt(DH))
    perm = np.arange(MPC)
    for h in range(HPC):
        base = h * DH
        perm[base:base + NROT:2] = np.arange(base + 1, base + NROT + 1, 2)
        perm[base + 1:base + NROT:2] = np.arange(base, base + NROT, 2)
    Wq, Wk, Wv, Wo = (np.asarray(a, f32) for a in (Wq, Wk, Wv, Wo))
    bq, bk, bv, bo = (np.asarray(a, f32) for a in (bq, bk, bv, bo))
    invf = _inv_freq_signed()
    if use_mask:
        maskT = np.ascontiguousarray((np.asarray(mask) > 0).astype(NPF16))

    in_maps = []
    for c in range(NCORES):
        sl = slice(c * MPC, (c + 1) * MPC)
        m = {
            "xqT": xqT, "xkvT": xkvT, "posq": posq, "posk": posk,
            "wqT": np.ascontiguousarray((scale * Wq[sl, :]).T).astype(NPF16),
            "wkT": np.ascontiguousarray(Wk[sl, :].T).astype(NPF16),
            "wvT": np.ascontiguousarray(Wv[sl, :].T).astype(NPF16),
            "wqTs": np.ascontiguousarray(
                (scale * Wq[sl, :][perm, :]).T).astype(NPF16),
            "wkTs": np.ascontiguousarray(Wk[sl, :][perm, :].T).astype(NPF16),
            "bq": (scale * bq[sl]).reshape(MPC, 1),
            "bk": bk[sl].reshape(MPC, 1).copy(),
            "bv": bv[sl].reshape(1, MPC).astype(NPF16),
            "bqs": (scale * bq[sl])[perm].reshape(MPC, 1).copy(),
            "bks": bk[sl][perm].reshape(MPC, 1).copy(),
            "woT0": np.ascontiguousarray(np.concatenate(
                [(bo if c == 0 else np.zeros_like(bo))[None, :],
                 np.zeros((DH - 1, D), f32),
                 Wo[:, c * MPC:c * MPC + DH].T], axis=0)).astype(NPF16),
            "woT1": np.ascontiguousarray(np.concatenate(
                [np.zeros((DH, D), f32),
                 Wo[:, c * MPC + DH:(c + 1) * MPC].T], axis=0)).astype(NPF16),
            "invf": (invf if (c + 1) * HPC <= NHEADS_ROT
                     else np.zeros_like(invf)).reshape(1, 128),
        }
        if use_mask:
            m["maskT"] = maskT
        in_maps.append(m)
    return in_maps


_CACHE = {}


def _get(use_mask):
    if use_mask not in _CACHE:
        nc = build_kernel(use_mask)
        _CACHE[use_mask] = (nc, _make_runner(nc))
    return _CACHE[use_mask]


def kernel(inputs_q, inputs_kv, mask, q_positions, kv_positions,
           Wq, bq, Wk, bk, Wv, bv, Wo, bo, _time_iters=0):
    use_mask = not bool(np.all(np.asarray(mask) > 0))
    nc, run = _get(use_mask)
    in_maps = make_in_maps(inputs_q, inputs_kv, mask, q_positions,
                           kv_positions, Wq, bq, Wk, bk, Wv, bv, Wo, bo,
                           use_mask)
    results, times = run(in_maps, time_iters=_time_iters)
    acc = np.zeros((D, T), np.float64)
    for c in range(NCORES):
        acc += results[c]["outT"]
    out = acc.astype(np.float32).reshape(D, B, TQ).transpose(2, 1, 0)
    out = np.ascontiguousarray(out)
    if _time_iters:
        kernel._last_times = times
    return out



# revision 40
# speedup vs baseline: 3.6788x; 1.6626x over previous
"""CrossAttention Trainium2 kernel — 8-core tensor-parallel (2 heads/core).

Self-contained: builds a Bass/Tile kernel, shards the full inputs across the
8 NeuronCores, runs via the axon PJRT path, and gathers the full output.

Per-core layout (core c owns heads 2c, 2c+1 = 128 of 1024 model dims):
  phase P: q/k/v projections (fp16 matmuls, fp32 PSUM) + rotary applied with
           host-precomputed cos/sin tables; the "spliced" operand comes from a
           partition-pair-swapped SBUF->SBUF DMA copy (sign folded into sin).
  phase A: per (b, qb, h): QK^T scores into 2-bank PSUM tiles, batched Exp on
           the Act engine, PV with q-tokens on PSUM partitions (out free = 65:
           64 channels + a ones-column giving the softmax denominator), then
           normalize on evacuation via a per-partition reciprocal scale.
  phase O: transpose attention output back to [chan, tok] via identity
           matmuls, single-pass output projection, partial fp16 outputs summed
           (+ Wo@bv + bo folded in) on the host.
"""

import sys
import time

for _p in ("/opt/trn_rl_repo", "/root/.axon_site/_ro/trn_rl_repo"):
    if _p not in sys.path:
        sys.path.insert(0, _p)

import numpy as np
from contextlib import ExitStack

import concourse.bacc as bacc
import concourse.mybir as mybir
import concourse.tile as tile
from concourse.mybir import ActivationFunctionType as AF
from concourse.mybir import AluOpType as ALU

# ---------------------------------------------------------------- problem dims
D = 1024
H = 16
DH = 64
TQ = 2048
TKV = 2048
B = 2
NCORES = 8
HPC = H // NCORES          # heads per core = 2
MPC = HPC * DH             # dims per core  = 128
T = B * TQ                 # token axis (b-grouped) = 4096
NROT = 32                  # rotated channels per head (frac 0.5 of 64)
NHEADS_ROT = 12            # rotated heads (frac 0.75 of 16)
MAX_WL = 8192.0

F32 = mybir.dt.float32
F16 = mybir.dt.float16
NPF16 = np.float16

KC = D // 128              # 8 contraction chunks for projections
TB = T // 512              # 8 token blocks of 512
TBB = TQ // 512            # 4 token blocks per batch
QB = TQ // 512             # 4 query blocks per batch
KVC = TKV // 128           # 16 kv chunks per batch
VW = 2 * (DH + 1)          # vn chunk width: [h0 ch, ones, h1 ch, ones] = 130


# ---------------------------------------------------------------- bass builder
def build_kernel(use_mask: bool):
    nc = bacc.Bacc("TRN2", target_bir_lowering=False, debug=False,
                   enable_asserts=True, num_devices=NCORES)

    xqT = nc.dram_tensor("xqT", [D, T], F16, kind="ExternalInput").ap()
    xkvT = nc.dram_tensor("xkvT", [D, T], F16, kind="ExternalInput").ap()
    wqT_d = nc.dram_tensor("wqT", [D, MPC], F16, kind="ExternalInput").ap()
    wkT_d = nc.dram_tensor("wkT", [D, MPC], F16, kind="ExternalInput").ap()
    wvT_d = nc.dram_tensor("wvT", [D, MPC], F16, kind="ExternalInput").ap()
    bq_d = nc.dram_tensor("bq", [MPC, 1], F32, kind="ExternalInput").ap()
    bk_d = nc.dram_tensor("bk", [MPC, 1], F32, kind="ExternalInput").ap()
    woT_d = nc.dram_tensor("woT", [MPC, D], F16, kind="ExternalInput").ap()
    cosq_d = nc.dram_tensor("cosq", [128, T], F16, kind="ExternalInput").ap()
    sinq_d = nc.dram_tensor("sinq", [128, T], F16, kind="ExternalInput").ap()
    cosk_d = nc.dram_tensor("cosk", [128, T], F16, kind="ExternalInput").ap()
    sink_d = nc.dram_tensor("sink", [128, T], F16, kind="ExternalInput").ap()
    iden_d = nc.dram_tensor("iden", [128, 128], F16, kind="ExternalInput").ap()
    if use_mask:
        mask_d = nc.dram_tensor("maskT", [TKV, B, TQ], F16,
                                kind="ExternalInput").ap()
    outT = nc.dram_tensor("outT", [D, T], F16, kind="ExternalOutput").ap()

    with tile.TileContext(nc) as tc:
        with ExitStack() as octx:
            persist = octx.enter_context(tc.tile_pool(name="persist", bufs=1))
            xio = octx.enter_context(tc.tile_pool(name="xio", bufs=16))
            rotp = octx.enter_context(tc.tile_pool(name="rotp", bufs=4))
            ptp = octx.enter_context(tc.tile_pool(name="ptp", bufs=16))
            smlp = octx.enter_context(tc.tile_pool(name="smlp", bufs=8))
            otqp = octx.enter_context(tc.tile_pool(name="otqp", bufs=16))
            osb = octx.enter_context(tc.tile_pool(name="osb", bufs=4))
            pjp = octx.enter_context(tc.tile_pool(name="pjp", bufs=2,
                                                  space="PSUM"))
            stp = octx.enter_context(tc.tile_pool(name="stp", bufs=2,
                                                  space="PSUM"))
            pvp = octx.enter_context(tc.tile_pool(name="pvp", bufs=2,
                                                  space="PSUM"))
            if use_mask:
                mkp = octx.enter_context(tc.tile_pool(name="mkp", bufs=8))

            # -------- persistent tensors
            qT = persist.tile([128, T], F16, tag="qT")
            kT = persist.tile([128, T], F16, tag="kT")
            ot_t = persist.tile([128, T], F16, tag="ot_t")
            cosq = persist.tile([128, T], F16, tag="cosq")
            sinq = persist.tile([128, T], F16, tag="sinq")
            cosk = persist.tile([128, T], F16, tag="cosk")
            sink = persist.tile([128, T], F16, tag="sink")
            vn = persist.tile([128, B * KVC * VW], F16, tag="vn")
            vn3 = vn.rearrange("p (c w) -> p c w", w=VW)
            woT = persist.tile([MPC, D], F16, tag="woT")
            iden = persist.tile([128, 128], F16, tag="iden")
            bq_sb = persist.tile([MPC, 1], F32, tag="bq")
            bk_sb = persist.tile([MPC, 1], F32, tag="bk")

            nc.sync.dma_start(woT[:], woT_d[:])
            nc.sync.dma_start(iden[:], iden_d[:])
            nc.sync.dma_start(bq_sb[:], bq_d[:])
            nc.sync.dma_start(bk_sb[:], bk_d[:])
            nc.sync.dma_start(cosq[:], cosq_d[:])
            nc.sync.dma_start(sinq[:], sinq_d[:])
            nc.sync.dma_start(cosk[:], cosk_d[:])
            nc.sync.dma_start(sink[:], sink_d[:])
            nc.vector.memset(vn3[:, :, DH:DH + 1], 1.0)
            nc.vector.memset(vn3[:, :, 2 * DH + 1:2 * DH + 2], 1.0)

            wq_sb, wk_sb, wv_sb = [], [], []
            for kc in range(KC):
                for lst, src, tg in ((wq_sb, wqT_d, "wq"), (wk_sb, wkT_d, "wk"),
                                     (wv_sb, wvT_d, "wv")):
                    t = persist.tile([128, MPC], F16, tag=f"{tg}{kc}",
                                     name=f"{tg}{kc}")
                    nc.sync.dma_start(t[:], src[kc * 128:(kc + 1) * 128, :])
                    lst.append(t)

            # -------- emit helpers ------------------------------------------
            def proj_block(tb):
                """projections + rotary for one 512-token block."""
                sl = slice(tb * 512, (tb + 1) * 512)
                xq_ts, xkv_ts = [], []
                for kc in range(KC):
                    xq_t = xio.tile([128, 512], F16, tag="xq", name="xq_t")
                    nc.sync.dma_start(xq_t[:], xqT[kc * 128:(kc + 1) * 128, sl])
                    xkv_t = xio.tile([128, 512], F16, tag="xkv", name="xkv_t")
                    nc.sync.dma_start(xkv_t[:],
                                      xkvT[kc * 128:(kc + 1) * 128, sl])
                    xq_ts.append(xq_t)
                    xkv_ts.append(xkv_t)
                q_ps = pjp.tile([128, 512], F32, tag="pj", name="q_ps")
                for kc in range(KC):
                    nc.tensor.matmul(q_ps[:], wq_sb[kc][:], xq_ts[kc][:],
                                     start=(kc == 0), stop=(kc == KC - 1))
                nc.vector.tensor_scalar(qT[:, sl], q_ps[:], bq_sb[:], None,
                                        ALU.add)
                k_ps = pjp.tile([128, 512], F32, tag="pj", name="k_ps")
                for kc in range(KC):
                    nc.tensor.matmul(k_ps[:], wk_sb[kc][:], xkv_ts[kc][:],
                                     start=(kc == 0), stop=(kc == KC - 1))
                nc.vector.tensor_scalar(kT[:, sl], k_ps[:], bk_sb[:], None,
                                        ALU.add)
                # v natural layout: per 128-token quarter [tok, chan]
                v_ps = pjp.tile([128, 512], F32, tag="pj", name="v_ps")
                for tc4 in range(4):
                    vsl = slice(tc4 * 128, (tc4 + 1) * 128)
                    for kc in range(KC):
                        nc.tensor.matmul(v_ps[:, vsl], xkv_ts[kc][:, vsl],
                                         wv_sb[kc][:], start=(kc == 0),
                                         stop=(kc == KC - 1))
                for tc4 in range(4):
                    g = tb * 4 + tc4
                    nc.vector.tensor_copy(vn3[:, g, 0:DH],
                                          v_ps[:, tc4 * 128:tc4 * 128 + DH])
                    nc.vector.tensor_copy(
                        vn3[:, g, DH + 1:2 * DH + 1],
                        v_ps[:, tc4 * 128 + DH:tc4 * 128 + 2 * DH])
                # rotary (sign folded into sin tables; swap via SBUF DMA)
                for xt, cs, sn in ((qT, cosq, sinq), (kT, cosk, sink)):
                    sh = rotp.tile([128, 512], F16, tag="sh", name="sh")
                    nc.sync.dma_start(sh[0:127:2, :], xt[1:128:2, sl])
                    nc.sync.dma_start(sh[1:128:2, :], xt[0:127:2, sl])
                    nc.vector.tensor_mul(sh[:], sh[:], sn[:, sl])
                    nc.vector.tensor_mul(xt[:, sl], xt[:, sl], cs[:, sl])
                    nc.vector.tensor_add(xt[:, sl], xt[:, sl], sh[:])

            otq_tiles = {}

            def qk_exp(b, qb, h):
                """scores + exp for one (batch, 512-query-block, head)."""
                hsl = slice(h * DH, (h + 1) * DH)
                qsl = slice(b * TQ + qb * 512, b * TQ + (qb + 1) * 512)
                mts = mask_tiles.get((b, qb)) if use_mask else None
                pts = []
                for half in range(KVC // 2):
                    st = stp.tile([128, 1024], F32, tag="st", name="st")
                    for j in range(2):
                        kc = half * 2 + j
                        nc.tensor.matmul(
                            st[:, j * 512:(j + 1) * 512],
                            kT[hsl, b * TKV + kc * 128:b * TKV + (kc + 1) * 128],
                            qT[hsl, qsl], start=True, stop=True)
                    pt = ptp.tile([128, 1024], F16, tag="pt", name="pt")
                    nc.scalar.activation(pt[:], st[:], AF.Exp)
                    if use_mask:
                        nc.vector.tensor_mul(pt[:], pt[:], mts[half][:])
                    pts.append(pt)
                return pts

            def pv_norm(b, qb, h, pts):
                """PV with q on partitions, denominator col, normalize+evac."""
                ov = pvp.tile([128, 4 * (DH + 1)], F32, tag="pv", name="ov")
                for qt in range(4):
                    osl = slice(qt * (DH + 1), (qt + 1) * (DH + 1))
                    for kc in range(KVC):
                        pcol = (kc % 2) * 512 + qt * 128
                        nc.tensor.matmul(
                            ov[:, osl],
                            pts[kc // 2][:, pcol:pcol + 128],
                            vn3[:, b * KVC + kc, h * (DH + 1):
                                (h + 1) * (DH + 1)],
                            start=(kc == 0), stop=(kc == KVC - 1))
                for qt in range(4):
                    base = qt * (DH + 1)
                    rec = smlp.tile([128, 1], F32, tag="rec", name="rec")
                    nc.vector.reciprocal(rec[:],
                                         ov[:, base + DH:base + DH + 1])
                    otq = otqp.tile([128, DH], F16, tag="otq", name="otq")
                    nc.vector.tensor_scalar(otq[:], ov[:, base:base + DH],
                                            rec[:], None, ALU.mult)
                    otq_tiles[(h, qt)] = otq

            def transposes(b, qb):
                """[q, chan] -> ot_t[chan, tok] via identity matmuls."""
                for qt in range(4):
                    tr = pvp.tile([128, 4 * (DH + 1)], F32, tag="pv",
                                  name="tr")
                    for h in range(HPC):
                        nc.tensor.matmul(tr[h * DH:(h + 1) * DH, 0:128],
                                         otq_tiles[(h, qt)][:], iden[:],
                                         start=True, stop=True)
                    col = b * TQ + qb * 512 + qt * 128
                    nc.vector.tensor_copy(ot_t[:, col:col + 128],
                                          tr[:, 0:128])

            def outproj_unit(b, jc, tb4):
                o_ps = pjp.tile([128, 512], F32, tag="pj", name="o_ps")
                col = b * TQ + tb4 * 512
                nc.tensor.matmul(o_ps[:], woT[:, jc * 128:(jc + 1) * 128],
                                 ot_t[:, col:col + 512], start=True, stop=True)
                o_sb = osb.tile([128, 512], F16, tag="o_sb", name="o_sb")
                nc.vector.tensor_copy(o_sb[:], o_ps[:])
                nc.sync.dma_start(outT[jc * 128:(jc + 1) * 128,
                                       col:col + 512], o_sb[:])

            mask_tiles = {}

            def load_mask(b, qb):
                if not use_mask or (b, qb) in mask_tiles:
                    return
                mts = []
                for half in range(KVC // 2):
                    mt = mkp.tile([128, 1024], F16, tag="mk", name="mt")
                    for j in range(2):
                        kc = half * 2 + j
                        nc.sync.dma_start(
                            mt[:, j * 512:(j + 1) * 512],
                            mask_d[kc * 128:(kc + 1) * 128, b,
                                   qb * 512:(qb + 1) * 512])
                    mts.append(mt)
                mask_tiles[(b, qb)] = mts

            # -------- schedule ----------------------------------------------
            # proj b0 fully + first block of b1 up front; remaining b1 blocks
            # and b0's output projection are interleaved into the attention
            # pipeline so the PE fills Act-bound gaps. One-combo-deep software
            # pipeline: QK+exp of combo i issues before PV of combo i-1.
            for tb4 in range(TBB):
                proj_block(0 * TBB + tb4)
            proj_block(1 * TBB + 0)

            combos = [(b, qb, h) for b in range(B) for qb in range(QB)
                      for h in range(HPC)]
            # filler PE work emitted between QK(i) and PV(i-1):
            fillers = {}
            for qb in range(3):
                # after combo (0, qb, 1) issues its QK
                fillers[(0, qb + 1, 0)] = [("proj", 1 * TBB + qb + 1)]
            # b0 outproj can only start once transposes(0,3) are emitted,
            # which happens during combo index 8 — so fill slots from 9 on.
            for i, (b, qb, h) in enumerate(combos):
                if i >= 9:
                    fillers.setdefault((b, qb, h), []).extend(
                        ("outproj", 0, jc, tb4)
                        for jc, tb4 in _op_units(i - 9))
            prev = None
            pts_prev = None
            for c in combos:
                load_mask(c[0], c[1])
                pts = qk_exp(*c)
                for f in fillers.get(c, []):
                    if f[0] == "proj":
                        proj_block(f[1])
                    else:
                        outproj_unit(f[1], f[2], f[3])
                if prev is not None:
                    pv_norm(*prev, pts_prev)
                    if prev[2] == HPC - 1:
                        transposes(prev[0], prev[1])
                prev, pts_prev = c, pts
            pv_norm(*prev, pts_prev)
            transposes(prev[0], prev[1])
            for jc, tb4 in _op_units(7):     # b0 remainder (jc=7)
                outproj_unit(0, jc, tb4)
            for i in range(8):
                for jc, tb4 in _op_units(i):
                    outproj_unit(1, jc, tb4)

    nc.compile()
    return nc


def _op_units(i):
    """4 output-projection (jc, tb4) units for slot i of 8."""
    out = []
    for k in range(4):
        u = i * 4 + k
        out.append((u // 4, u % 4))
    return out


# ---------------------------------------------------------------- pjrt runner
def _make_runner(nc, n_cores=NCORES):
    import jax
    from jax.sharding import Mesh, PartitionSpec
    from jax.experimental.shard_map import shard_map
    from concourse.bass2jax import (_bass_exec_p, install_neuronx_cc_hook,
                                    partition_id_tensor)

    install_neuronx_cc_hook()
    partition_name = (nc.partition_id_tensor.name
                      if nc.partition_id_tensor else None)
    in_names, out_names, out_avals, zero_shapes = [], [], [], []
    for alloc in nc.m.functions[0].allocations:
        if not isinstance(alloc, mybir.MemoryLocationSet):
            continue
        name = alloc.memorylocations[0].name
        if alloc.kind == "ExternalInput":
            if name != partition_name:
                in_names.append(name)
        elif alloc.kind == "ExternalOutput":
            shape = tuple(alloc.tensor_shape)
            dtype = mybir.dt.np(alloc.dtype)
            out_names.append(name)
            out_avals.append(jax.core.ShapedArray(shape, dtype))
            zero_shapes.append((shape, dtype))
    n_params = len(in_names)
    n_outs = len(out_avals)
    all_in_names = list(in_names) + list(out_names)
    if partition_name is not None:
        all_in_names.append(partition_name)

    def _body(*args):
        operands = list(args)
        if partition_name is not None:
            operands.append(partition_id_tensor())
        return tuple(_bass_exec_p.bind(
            *operands, out_avals=tuple(out_avals), in_names=tuple(all_in_names),
            out_names=tuple(out_names), lowering_input_output_aliases=(),
            sim_require_finite=True, sim_require_nnan=True, nc=nc))

    devices = jax.devices()[:n_cores]
    mesh = Mesh(np.asarray(devices), ("core",))
    in_specs = (PartitionSpec("core"),) * (n_params + n_outs)
    out_specs = (PartitionSpec("core"),) * len(out_names)
    donate = tuple(range(n_params, n_params + n_outs))
    sharded = jax.jit(
        shard_map(_body, mesh=mesh, in_specs=in_specs, out_specs=out_specs,
                  check_rep=False),
        donate_argnums=donate, keep_unused=True)

    def run(in_maps, time_iters=0):
        per_core = [[np.asarray(m[name]) for name in in_names]
                    for m in in_maps]
        concat_in = [np.concatenate([per_core[c][i] for c in range(n_cores)],
                                    axis=0) for i in range(n_params)]

        def zeros():
            return [np.zeros((n_cores * s[0], *s[1:]), d)
                    for s, d in zero_shapes]

        import jax
        out_arrs = sharded(*concat_in, *zeros())
        jax.block_until_ready(out_arrs)
        times = []
        for _ in range(time_iters):
            t0 = time.perf_counter()
            o = sharded(*concat_in, *zeros())
            jax.block_until_ready(o)
            times.append(time.perf_counter() - t0)
            out_arrs = o
        results = [
            {name: np.asarray(out_arrs[i]).reshape(n_cores,
                                                   *out_avals[i].shape)[c]
             for i, name in enumerate(out_names)}
            for c in range(n_cores)]
        return results, times

    return run


# ---------------------------------------------------------------- host shard
def _tables(positions, core, npos_dtype=np.float64):
    """cos/sin [128, T] fp16 tables; sign of the splice folded into sin."""
    pos = np.asarray(positions, np.float64).T.reshape(T)   # b-major tokens
    nb = NROT // 2
    freq = MAX_WL ** (2.0 / NROT * np.linspace(0.0, float(nb), nb))
    inv = 1.0 / freq                                        # [16]
    cos = np.ones((128, T), np.float64)
    sin = np.zeros((128, T), np.float64)
    for hl in range(HPC):
        hglob = core * HPC + hl
        if hglob >= NHEADS_ROT:
            continue
        for cc in range(NROT):
            ang = pos * inv[cc // 2]
            r = hl * DH + cc
            cos[r] = np.cos(ang)
            sgn = -1.0 if cc % 2 == 0 else 1.0
            sin[r] = sgn * np.sin(ang)
    return cos.astype(NPF16), sin.astype(NPF16)


def make_in_maps(inputs_q, inputs_kv, mask, q_positions, kv_positions,
                 Wq, bq, Wk, bk, Wv, bv, Wo, bo, use_mask):
    f32 = np.float32
    xqT = np.ascontiguousarray(
        np.asarray(inputs_q, f32).transpose(2, 1, 0).reshape(D, T)).astype(NPF16)
    xkvT = np.ascontiguousarray(
        np.asarray(inputs_kv, f32).transpose(2, 1, 0).reshape(D, T)).astype(NPF16)
    scale = f32(1.0 / np.sqrt(DH))
    Wq, Wk, Wv, Wo = (np.asarray(a, f32) for a in (Wq, Wk, Wv, Wo))
    bq, bk, bv, bo = (np.asarray(a, f32) for a in (bq, bk, bv, bo))
    iden = np.eye(128, dtype=NPF16)
    if use_mask:
        maskT = np.ascontiguousarray((np.asarray(mask) > 0).astype(NPF16))

    in_maps = []
    for c in range(NCORES):
        sl = slice(c * MPC, (c + 1) * MPC)
        cq, sq = _tables(q_positions, c)
        ck, sk = _tables(kv_positions, c)
        m = {
            "xqT": xqT, "xkvT": xkvT,
            "wqT": np.ascontiguousarray((scale * Wq[sl, :]).T).astype(NPF16),
            "wkT": np.ascontiguousarray(Wk[sl, :].T).astype(NPF16),
            "wvT": np.ascontiguousarray(Wv[sl, :].T).astype(NPF16),
            "bq": (scale * bq[sl]).reshape(MPC, 1),
            "bk": bk[sl].reshape(MPC, 1).copy(),
            "woT": np.ascontiguousarray(Wo[:, sl].T).astype(NPF16),
            "cosq": cq, "sinq": sq, "cosk": ck, "sink": sk,
            "iden": iden,
        }
        if use_mask:
            m["maskT"] = maskT
        in_maps.append(m)
    return in_maps


_CACHE = {}


def _get(use_mask):
    if use_mask not in _CACHE:
        nc = build_kernel(use_mask)
        _CACHE[use_mask] = (nc, _make_runner(nc))
    return _CACHE[use_mask]


def kernel(inputs_q, inputs_kv, mask, q_positions, kv_positions,
           Wq, bq, Wk, bk, Wv, bv, Wo, bo, _time_iters=0):
    use_mask = not bool(np.all(np.asarray(mask) > 0))
    nc, run = _get(use_mask)
    in_maps = make_in_maps(inputs_q, inputs_kv, mask, q_positions,
                           kv_positions, Wq, bq, Wk, bk, Wv, bv, Wo, bo,
                           use_mask)
    results, times = run(in_maps, time_iters=_time_iters)
    acc = np.zeros((D, T), np.float64)
    for c in range(NCORES):
        acc += results[c]["outT"].astype(np.float64)
    bo_full = (np.asarray(Wo, np.float64) @ np.asarray(bv, np.float64)
               + np.asarray(bo, np.float64))
    acc += bo_full[:, None]
    out = acc.astype(np.float32).reshape(D, B, TQ).transpose(2, 1, 0)
    out = np.ascontiguousarray(out)
    if _time_iters:
        kernel._last_times = times
    return out


# revision 77
# speedup vs baseline: 5.1500x; 1.3999x over previous
"""CrossAttention Trainium2 kernel — 8-core tensor-parallel (2 heads/core).

Self-contained: builds a Bass/Tile kernel, shards the full inputs across the
8 NeuronCores, runs via the axon PJRT path, and gathers the full output.

Per-core layout (core c owns heads 2c, 2c+1 = 128 of 1024 model dims):
  phase P: q/k/v projections (fp16 matmuls, fp32 PSUM) + rotary applied with
           host-precomputed cos/sin tables; the "spliced" operand comes from a
           partition-pair-swapped SBUF->SBUF DMA copy (sign folded into sin).
  phase A: per (b, qb, h): QK^T scores into 2-bank PSUM tiles, batched Exp on
           the Act engine, PV with q-tokens on PSUM partitions (out free = 65:
           64 channels + a ones-column giving the softmax denominator), then
           normalize on evacuation via a per-partition reciprocal scale.
  phase O: transpose attention output back to [chan, tok] via identity
           matmuls, single-pass output projection, partial fp16 outputs summed
           (+ Wo@bv + bo folded in) on the host.
"""

import sys
import time

for _p in ("/opt/trn_rl_repo", "/root/.axon_site/_ro/trn_rl_repo"):
    if _p not in sys.path:
        sys.path.insert(0, _p)

import numpy as np
from contextlib import ExitStack

import concourse.bacc as bacc
import concourse.mybir as mybir
import concourse.tile as tile
from concourse.mybir import ActivationFunctionType as AF
from concourse.mybir import AluOpType as ALU

# ---------------------------------------------------------------- problem dims
D = 1024
H = 16
DH = 64
TQ = 2048
TKV = 2048
B = 2
NCORES = 8
HPC = H // NCORES          # heads per core = 2
MPC = HPC * DH             # dims per core  = 128
T = B * TQ                 # token axis (b-grouped) = 4096
NROT = 32                  # rotated channels per head (frac 0.5 of 64)
NHEADS_ROT = 12            # rotated heads (frac 0.75 of 16)
MAX_WL = 8192.0

F32 = mybir.dt.float32
F16 = mybir.dt.float16
NPF16 = np.float16

KC = D // 128              # 8 contraction chunks for projections
TB = T // 512              # 8 token blocks of 512
TBB = TQ // 512            # 4 token blocks per batch
QB = TQ // 512             # 4 query blocks per batch
KVC = TKV // 128           # 16 kv chunks per batch
VW = 2 * (DH + 1)          # vn chunk width: [h0 ch, ones, h1 ch, ones] = 130


# ---------------------------------------------------------------- bass builder
def build_kernel(use_mask: bool):
    nc = bacc.Bacc("TRN2", target_bir_lowering=False, debug=False,
                   enable_asserts=True, num_devices=NCORES)

    xqT = nc.dram_tensor("xqT", [D, T], F16, kind="ExternalInput").ap()
    xkvT = nc.dram_tensor("xkvT", [D, T], F16, kind="ExternalInput").ap()
    wqT_d = nc.dram_tensor("wqT", [D, MPC], F16, kind="ExternalInput").ap()
    wkT_d = nc.dram_tensor("wkT", [D, MPC], F16, kind="ExternalInput").ap()
    wvT_d = nc.dram_tensor("wvT", [D, MPC], F16, kind="ExternalInput").ap()
    bq_d = nc.dram_tensor("bq", [MPC, 1], F32, kind="ExternalInput").ap()
    bk_d = nc.dram_tensor("bk", [MPC, 1], F32, kind="ExternalInput").ap()
    woT_d = nc.dram_tensor("woT", [MPC, D], F16, kind="ExternalInput").ap()
    cosq_d = nc.dram_tensor("cosq", [128, T], F16, kind="ExternalInput").ap()
    sinq_d = nc.dram_tensor("sinq", [128, T], F16, kind="ExternalInput").ap()
    cosk_d = nc.dram_tensor("cosk", [128, T], F16, kind="ExternalInput").ap()
    sink_d = nc.dram_tensor("sink", [128, T], F16, kind="ExternalInput").ap()
    iden_d = nc.dram_tensor("iden", [128, 128], F16, kind="ExternalInput").ap()
    if use_mask:
        mask_d = nc.dram_tensor("maskT", [TKV, B, TQ], F16,
                                kind="ExternalInput").ap()
    outT = nc.dram_tensor("outT", [D, T], F16, kind="ExternalOutput").ap()

    with tile.TileContext(nc) as tc:
        with ExitStack() as octx:
            persist = octx.enter_context(tc.tile_pool(name="persist", bufs=1))
            xio = octx.enter_context(tc.tile_pool(name="xio", bufs=2))
            rotp = octx.enter_context(tc.tile_pool(name="rotp", bufs=4))
            ptp = octx.enter_context(tc.tile_pool(name="ptp", bufs=16))
            smlp = octx.enter_context(tc.tile_pool(name="smlp", bufs=8))
            otqp = octx.enter_context(tc.tile_pool(name="otqp", bufs=16))
            osb = octx.enter_context(tc.tile_pool(name="osb", bufs=10))
            pjp = octx.enter_context(tc.tile_pool(name="pjp", bufs=2,
                                                  space="PSUM"))
            stp = octx.enter_context(tc.tile_pool(name="stp", bufs=2,
                                                  space="PSUM"))
            pvp = octx.enter_context(tc.tile_pool(name="pvp", bufs=2,
                                                  space="PSUM"))
            if use_mask:
                mkp = octx.enter_context(tc.tile_pool(name="mkp", bufs=8))

            # -------- persistent tensors
            qT = persist.tile([128, T], F16, tag="qT")
            kT = persist.tile([128, T], F16, tag="kT")
            ot_t = persist.tile([128, T], F16, tag="ot_t")
            cosq = persist.tile([128, T], F16, tag="cosq")
            sinq = persist.tile([128, T], F16, tag="sinq")
            cosk = persist.tile([128, T], F16, tag="cosk")
            sink = persist.tile([128, T], F16, tag="sink")
            # vn chunk layout [ones | h0 chans | h1 chans | ones] so v-evac is
            # a single [128,128] copy and both heads' PV moving APs (chans +
            # their ones column) stay contiguous.
            vn = persist.tile([128, B * KVC * VW], F16, tag="vn")
            vn3 = vn.rearrange("p (c w) -> p c w", w=VW)
            woT = persist.tile([MPC, D], F16, tag="woT")
            iden = persist.tile([128, 128], F16, tag="iden")
            bq_sb = persist.tile([MPC, 1], F32, tag="bq")
            bk_sb = persist.tile([MPC, 1], F32, tag="bk")
            wq_all = persist.tile([128, KC, MPC], F16, tag="wq_all")
            wk_all = persist.tile([128, KC, MPC], F16, tag="wk_all")
            wv_all = persist.tile([128, KC, MPC], F16, tag="wv_all")

            # kv-side weights + first xkv block + b0-half k tables only: the
            # SP DMA queue is kept in just-in-time order so the startup
            # critical path (kv0, kv1, q0, first QK halves) is fed first.
            # Everything else is emitted later, interleaved into the
            # schedule.
            nc.sync.dma_start(wk_all[:],
                              wkT_d.rearrange("(c p) m -> p c m", p=128))
            nc.sync.dma_start(wv_all[:],
                              wvT_d.rearrange("(c p) m -> p c m", p=128))
            nc.sync.dma_start(bk_sb[:], bk_d[:])
            xq3 = xqT.rearrange("(c p) t -> p c t", p=128)
            xkv3 = xkvT.rearrange("(c p) t -> p c t", p=128)
            pre0kv = xio.tile([128, KC, 512], F16, tag="xkv", name="xkv_t0")
            nc.sync.dma_start(pre0kv[:], xkv3[:, :, 0:512])
            nc.sync.dma_start(cosk[:, 0:TQ], cosk_d[:, 0:TQ])
            nc.sync.dma_start(sink[:, 0:TQ], sink_d[:, 0:TQ])
            nc.vector.memset(vn3[:, :, 0:1], 1.0)
            nc.vector.memset(vn3[:, :, 2 * DH + 1:2 * DH + 2], 1.0)
            wq_sb = [wq_all[:, kc, :] for kc in range(KC)]
            wk_sb = [wk_all[:, kc, :] for kc in range(KC)]
            wv_sb = [wv_all[:, kc, :] for kc in range(KC)]

            # -------- emit helpers ------------------------------------------
            def _rotary(xt, cs, sn, sl, dma=None):
                """x = x*cos + pairswap(x)*sin; swap via SBUF->SBUF DMA,
                splice sign folded into the sin table. Startup blocks issue
                the swap from Act (idle until attention starts) so the
                compute-dependent copies don't head-of-line block SP's
                input-load queue."""
                dma = dma or nc.sync
                sh = rotp.tile([128, 512], F16, tag="sh", name="sh")
                dma.dma_start(sh[0:127:2, :], xt[1:128:2, sl])
                dma.dma_start(sh[1:128:2, :], xt[0:127:2, sl])
                nc.vector.tensor_mul(sh[:], sh[:], sn[:, sl])
                nc.vector.tensor_mul(xt[:, sl], xt[:, sl], cs[:, sl])
                nc.vector.tensor_add(xt[:, sl], xt[:, sl], sh[:])

            xkv_tiles = {}

            def _xkv(tb, pre=None):
                if tb not in xkv_tiles:
                    if pre is not None:
                        xkv_tiles[tb] = pre
                    else:
                        t = xio.tile([128, KC, 512], F16, tag="xkv",
                                     name="xkv_t")
                        nc.sync.dma_start(t[:],
                                          xkv3[:, :, tb * 512:(tb + 1) * 512])
                        xkv_tiles[tb] = t
                return xkv_tiles[tb]

            def kvk_part(tb, pre=None, dma=None):
                """k projection + k rotary for one 512-token block."""
                sl = slice(tb * 512, (tb + 1) * 512)
                xkv_t = _xkv(tb, pre)
                k_ps = pjp.tile([128, 512], F32, tag="pj", name="k_ps")
                for kc in range(KC):
                    nc.tensor.matmul(k_ps[:], wk_sb[kc], xkv_t[:, kc, :],
                                     start=(kc == 0), stop=(kc == KC - 1))
                nc.vector.tensor_scalar(kT[:, sl], k_ps[:], bk_sb[:], None,
                                        ALU.add)
                _rotary(kT, cosk, sink, sl, dma)

            def kvv_part(tb):
                """v projection for one block; natural [tok, chan] layout."""
                xkv_t = _xkv(tb)
                v_ps = pjp.tile([128, 512], F32, tag="pj", name="v_ps")
                for tc4 in range(4):
                    vsl = slice(tc4 * 128, (tc4 + 1) * 128)
                    for kc in range(KC):
                        nc.tensor.matmul(v_ps[:, vsl], xkv_t[:, kc, vsl],
                                         wv_sb[kc], start=(kc == 0),
                                         stop=(kc == KC - 1))
                for tc4 in range(4):
                    g = tb * 4 + tc4
                    nc.vector.tensor_copy(vn3[:, g, 1:2 * DH + 1],
                                          v_ps[:, tc4 * 128:(tc4 + 1) * 128])
                del xkv_tiles[tb]

            def kv_part(tb, pre=None, dma=None):
                kvk_part(tb, pre, dma)
                kvv_part(tb)

            def q_part(tb, pre=None, dma=None):
                """q projection + rotary for one 512-token block."""
                sl = slice(tb * 512, (tb + 1) * 512)
                if pre is None:
                    xq_t = xio.tile([128, KC, 512], F16, tag="xq",
                                    name="xq_t")
                    nc.sync.dma_start(xq_t[:], xq3[:, :, sl])
                else:
                    xq_t = pre
                q_ps = pjp.tile([128, 512], F32, tag="pj", name="q_ps")
                for kc in range(KC):
                    nc.tensor.matmul(q_ps[:], wq_sb[kc], xq_t[:, kc, :],
                                     start=(kc == 0), stop=(kc == KC - 1))
                nc.vector.tensor_scalar(qT[:, sl], q_ps[:], bq_sb[:], None,
                                        ALU.add)
                _rotary(qT, cosq, sinq, sl, dma)

            otq_tiles = {}

            def qk_exp(b, qb, h, halves=None, pts=None):
                """scores + exp for one (batch, 512-query-block, head)."""
                hsl = slice(h * DH, (h + 1) * DH)
                qsl = slice(b * TQ + qb * 512, b * TQ + (qb + 1) * 512)
                mts = mask_tiles.get((b, qb)) if use_mask else None
                if pts is None:
                    pts = []
                for half in (range(KVC // 2) if halves is None else halves):
                    st = stp.tile([128, 1024], F32, tag="st", name="st")
                    for j in range(2):
                        kc = half * 2 + j
                        nc.tensor.matmul(
                            st[:, j * 512:(j + 1) * 512],
                            kT[hsl, b * TKV + kc * 128:b * TKV + (kc + 1) * 128],
                            qT[hsl, qsl], start=True, stop=True)
                    pt = ptp.tile([128, 1024], F16, tag="pt", name="pt")
                    nc.scalar.activation(pt[:], st[:], AF.Exp)
                    if use_mask:
                        nc.vector.tensor_mul(pt[:], pt[:], mts[half][:])
                    pts.append(pt)
                return pts

            def pv_norm(b, qb, h, pts):
                """PV with q on partitions, denominator col, normalize+evac."""
                ov = pvp.tile([128, 4 * (DH + 1)], F32, tag="pv", name="ov")
                for qt in range(4):
                    osl = slice(qt * (DH + 1), (qt + 1) * (DH + 1))
                    for kc in range(KVC):
                        pcol = (kc % 2) * 512 + qt * 128
                        nc.tensor.matmul(
                            ov[:, osl],
                            pts[kc // 2][:, pcol:pcol + 128],
                            vn3[:, b * KVC + kc, h * (DH + 1):
                                (h + 1) * (DH + 1)],
                            start=(kc == 0), stop=(kc == KVC - 1))
                s_off = 0 if h == 0 else DH       # ones col position per head
                c_off = 1 if h == 0 else 0
                for qt in range(4):
                    base = qt * (DH + 1)
                    rec = smlp.tile([128, 1], F32, tag="rec", name="rec")
                    nc.vector.reciprocal(
                        rec[:], ov[:, base + s_off:base + s_off + 1])
                    otq = otqp.tile([128, DH], F16, tag="otq", name="otq")
                    nc.vector.tensor_scalar(
                        otq[:], ov[:, base + c_off:base + c_off + DH],
                        rec[:], None, ALU.mult)
                    otq_tiles[(h, qt)] = otq

            def transposes(b, qb):
                """[q, chan] -> ot_t[chan, tok] via identity matmuls."""
                for qt in range(4):
                    tr = pvp.tile([128, 4 * (DH + 1)], F32, tag="pv",
                                  name="tr")
                    for h in range(HPC):
                        nc.tensor.matmul(tr[h * DH:(h + 1) * DH, 0:128],
                                         otq_tiles[(h, qt)][:], iden[:],
                                         start=True, stop=True)
                    col = b * TQ + qb * 512 + qt * 128
                    nc.vector.tensor_copy(ot_t[:, col:col + 128],
                                          tr[:, 0:128])

            osb_tiles = {}

            def outproj_quarter(b, jc, tb4, evac_act=False):
                """one [128,512] token-quarter of output row-block jc."""
                key = (b, jc)
                if key not in osb_tiles:
                    osb_tiles[key] = osb.tile([128, 2048], F16, tag="o_sb",
                                              name="o_sb")
                o_big = osb_tiles[key]
                o_ps = pjp.tile([128, 512], F32, tag="pj", name="o_ps")
                col = b * TQ + tb4 * 512
                nc.tensor.matmul(o_ps[:], woT[:, jc * 128:(jc + 1) * 128],
                                 ot_t[:, col:col + 512], start=True, stop=True)
                c0 = tb4 * 512
                if evac_act:     # tail quarters: Act is idle after last exp
                    nc.scalar.activation(o_big[:, c0:c0 + 512], o_ps[:],
                                         AF.Identity)
                else:
                    nc.vector.tensor_copy(o_big[:, c0:c0 + 512], o_ps[:])
                nc.gpsimd.dma_start(
                    outT[jc * 128:(jc + 1) * 128,
                         b * TQ + c0:b * TQ + c0 + 512],
                    o_big[:, c0:c0 + 512])

            mask_tiles = {}

            def load_mask(b, qb):
                if not use_mask or (b, qb) in mask_tiles:
                    return
                mts = []
                for half in range(KVC // 2):
                    mt = mkp.tile([128, 1024], F16, tag="mk", name="mt")
                    for j in range(2):
                        kc = half * 2 + j
                        nc.sync.dma_start(
                            mt[:, j * 512:(j + 1) * 512],
                            mask_d[kc * 128:(kc + 1) * 128, b,
                                   qb * 512:(qb + 1) * 512])
                    mts.append(mt)
                mask_tiles[(b, qb)] = mts

            # -------- schedule ----------------------------------------------
            # kv-side of b0 projects first and the first combo's QK halves
            # interleave with the remaining kv blocks, so exp starts as soon
            # as kv blocks 0-1 + q block 0 are rotated. The remaining q/kv
            # blocks and the output projection are interleaved into the
            # attention pipeline as PE fillers inside Act-bound exp windows.
            # One-combo-deep software pipeline: QK+exp of combo i issues
            # before PV of combo i-1. DMAs are emitted just-in-time so SP's
            # in-order queue feeds the startup critical path first.
            kv_part(0, pre=pre0kv, dma=nc.scalar)
            nc.sync.dma_start(wq_all[:],
                              wqT_d.rearrange("(c p) m -> p c m", p=128))
            nc.sync.dma_start(bq_sb[:], bq_d[:])
            nc.sync.dma_start(cosq[:, 0:TQ], cosq_d[:, 0:TQ])
            nc.sync.dma_start(sinq[:, 0:TQ], sinq_d[:, 0:TQ])
            kv_part(1, dma=nc.scalar)
            q_part(0, dma=nc.scalar)
            load_mask(0, 0)
            pts0 = qk_exp(0, 0, 0, halves=[0, 1, 2, 3])
            nc.sync.dma_start(iden[:], iden_d[:])
            kv_part(2, dma=nc.scalar)
            qk_exp(0, 0, 0, halves=[4, 5], pts=pts0)
            kv_part(3, dma=nc.scalar)
            nc.sync.dma_start(cosk[:, TQ:T], cosk_d[:, TQ:T])
            nc.sync.dma_start(sink[:, TQ:T], sink_d[:, TQ:T])
            qk_exp(0, 0, 0, halves=[6, 7], pts=pts0)
            nc.sync.dma_start(cosq[:, TQ:T], cosq_d[:, TQ:T])
            nc.sync.dma_start(sinq[:, TQ:T], sinq_d[:, TQ:T])
            nc.sync.dma_start(woT[:], woT_d[:])

            combos = [(b, qb, h) for b in range(B) for qb in range(QB)
                      for h in range(HPC)]
            fillers = {
                (0, 0, 1): [("q", 1)],
                (0, 1, 0): [("kv", 4)], (0, 1, 1): [("q", 2)],
                (0, 2, 0): [("kv", 5)], (0, 2, 1): [("q", 3), ("q", 4)],
                (0, 3, 0): [("kv", 6)], (0, 3, 1): [("kv", 7)],
                (1, 0, 0): [("q", 5)],
                (1, 1, 0): [("q", 6)],
                (1, 2, 0): [("q", 7)],
            }
            # outproj quarters: (0,*) available from combo index 9 (after
            # transposes(0,3) at index 8); (1,*,tb4) needs transposes(1,tb4),
            # emitted during combo index 10+2*tb4.
            opq = {9: [(0, 0), (0, 1)], 10: [(0, 2), (0, 3)],
                   11: [(0, 4), (0, 5)], 12: [(0, 6), (0, 7)]}
            for i, jcs in opq.items():
                fillers.setdefault(combos[i], []).extend(
                    ("op", b, jc, tb4) for b, jc in jcs for tb4 in range(4))
            h0slots = {13: [0, 1, 2], 14: [3, 4, 5], 15: [6, 7]}
            for i, jcs in h0slots.items():
                fillers.setdefault(combos[i], []).extend(
                    ("op", 1, jc, tb4) for jc in jcs for tb4 in (0, 1))
            # t2 quarters fit in the final exp window (transposes(1,2) are
            # emitted during combo 14)
            fillers.setdefault(combos[15], []).extend(
                ("op", 1, jc, 2) for jc in range(8))
            prev = (0, 0, 0)
            pts_prev = pts0
            for c in combos[1:]:
                load_mask(c[0], c[1])
                pts = qk_exp(*c)
                for f in fillers.get(c, []):
                    if f[0] == "q":
                        q_part(f[1])
                    elif f[0] == "kv":
                        kv_part(f[1])
                    else:
                        outproj_quarter(f[1], f[2], f[3])
                if prev is not None:
                    pv_norm(*prev, pts_prev)
                    if prev[2] == HPC - 1:
                        transposes(prev[0], prev[1])
                prev, pts_prev = c, pts
            pv_norm(*prev, pts_prev)
            transposes(prev[0], prev[1])
            for jc in range(8):
                outproj_quarter(1, jc, 3, evac_act=True)

    nc.compile()
    return nc


# ---------------------------------------------------------------- pjrt runner
def _make_runner(nc, n_cores=NCORES):
    import jax
    from jax.sharding import Mesh, PartitionSpec
    from jax.experimental.shard_map import shard_map
    from concourse.bass2jax import (_bass_exec_p, install_neuronx_cc_hook,
                                    partition_id_tensor)

    install_neuronx_cc_hook()
    partition_name = (nc.partition_id_tensor.name
                      if nc.partition_id_tensor else None)
    in_names, out_names, out_avals, zero_shapes = [], [], [], []
    for alloc in nc.m.functions[0].allocations:
        if not isinstance(alloc, mybir.MemoryLocationSet):
            continue
        name = alloc.memorylocations[0].name
        if alloc.kind == "ExternalInput":
            if name != partition_name:
                in_names.append(name)
        elif alloc.kind == "ExternalOutput":
            shape = tuple(alloc.tensor_shape)
            dtype = mybir.dt.np(alloc.dtype)
            out_names.append(name)
            out_avals.append(jax.core.ShapedArray(shape, dtype))
            zero_shapes.append((shape, dtype))
    n_params = len(in_names)
    n_outs = len(out_avals)
    all_in_names = list(in_names) + list(out_names)
    if partition_name is not None:
        all_in_names.append(partition_name)

    def _body(*args):
        operands = list(args)
        if partition_name is not None:
            operands.append(partition_id_tensor())
        return tuple(_bass_exec_p.bind(
            *operands, out_avals=tuple(out_avals), in_names=tuple(all_in_names),
            out_names=tuple(out_names), lowering_input_output_aliases=(),
            sim_require_finite=True, sim_require_nnan=True, nc=nc))

    devices = jax.devices()[:n_cores]
    mesh = Mesh(np.asarray(devices), ("core",))
    in_specs = (PartitionSpec("core"),) * (n_params + n_outs)
    out_specs = (PartitionSpec("core"),) * len(out_names)
    donate = tuple(range(n_params, n_params + n_outs))
    sharded = jax.jit(
        shard_map(_body, mesh=mesh, in_specs=in_specs, out_specs=out_specs,
                  check_rep=False),
        donate_argnums=donate, keep_unused=True)

    def run(in_maps, time_iters=0):
        per_core = [[np.asarray(m[name]) for name in in_names]
                    for m in in_maps]
        concat_in = [np.concatenate([per_core[c][i] for c in range(n_cores)],
                                    axis=0) for i in range(n_params)]

        def zeros():
            return [np.zeros((n_cores * s[0], *s[1:]), d)
                    for s, d in zero_shapes]

        import jax
        out_arrs = sharded(*concat_in, *zeros())
        jax.block_until_ready(out_arrs)
        times = []
        for _ in range(time_iters):
            t0 = time.perf_counter()
            o = sharded(*concat_in, *zeros())
            jax.block_until_ready(o)
            times.append(time.perf_counter() - t0)
            out_arrs = o
        results = [
            {name: np.asarray(out_arrs[i]).reshape(n_cores,
                                                   *out_avals[i].shape)[c]
             for i, name in enumerate(out_names)}
            for c in range(n_cores)]
        return results, times

    return run


# ---------------------------------------------------------------- host shard
def _tables(positions, core, npos_dtype=np.float64):
    """cos/sin [128, T] fp16 tables; sign of the splice folded into sin."""
    pos = np.asarray(positions, np.float64).T.reshape(T)   # b-major tokens
    nb = NROT // 2
    freq = MAX_WL ** (2.0 / NROT * np.linspace(0.0, float(nb), nb))
    inv = 1.0 / freq                                        # [16]
    cos = np.ones((128, T), np.float64)
    sin = np.zeros((128, T), np.float64)
    for hl in range(HPC):
        hglob = core * HPC + hl
        if hglob >= NHEADS_ROT:
            continue
        for cc in range(NROT):
            ang = pos * inv[cc // 2]
            r = hl * DH + cc
            cos[r] = np.cos(ang)
            sgn = -1.0 if cc % 2 == 0 else 1.0
            sin[r] = sgn * np.sin(ang)
    return cos.astype(NPF16), sin.astype(NPF16)


def make_in_maps(inputs_q, inputs_kv, mask, q_positions, kv_positions,
                 Wq, bq, Wk, bk, Wv, bv, Wo, bo, use_mask):
    f32 = np.float32
    xqT = np.ascontiguousarray(
        np.asarray(inputs_q, f32).transpose(2, 1, 0).reshape(D, T)).astype(NPF16)
    xkvT = np.ascontiguousarray(
        np.asarray(inputs_kv, f32).transpose(2, 1, 0).reshape(D, T)).astype(NPF16)
    scale = f32(1.0 / np.sqrt(DH))
    Wq, Wk, Wv, Wo = (np.asarray(a, f32) for a in (Wq, Wk, Wv, Wo))
    bq, bk, bv, bo = (np.asarray(a, f32) for a in (bq, bk, bv, bo))
    iden = np.eye(128, dtype=NPF16)
    if use_mask:
        maskT = np.ascontiguousarray((np.asarray(mask) > 0).astype(NPF16))

    in_maps = []
    for c in range(NCORES):
        sl = slice(c * MPC, (c + 1) * MPC)
        cq, sq = _tables(q_positions, c)
        ck, sk = _tables(kv_positions, c)
        m = {
            "xqT": xqT, "xkvT": xkvT,
            "wqT": np.ascontiguousarray((scale * Wq[sl, :]).T).astype(NPF16),
            "wkT": np.ascontiguousarray(Wk[sl, :].T).astype(NPF16),
            "wvT": np.ascontiguousarray(Wv[sl, :].T).astype(NPF16),
            "bq": (scale * bq[sl]).reshape(MPC, 1),
            "bk": bk[sl].reshape(MPC, 1).copy(),
            "woT": np.ascontiguousarray(Wo[:, sl].T).astype(NPF16),
            "cosq": cq, "sinq": sq, "cosk": ck, "sink": sk,
            "iden": iden,
        }
        if use_mask:
            m["maskT"] = maskT
        in_maps.append(m)
    return in_maps


_CACHE = {}


def _get(use_mask):
    if use_mask not in _CACHE:
        nc = build_kernel(use_mask)
        _CACHE[use_mask] = (nc, _make_runner(nc))
    return _CACHE[use_mask]


def kernel(inputs_q, inputs_kv, mask, q_positions, kv_positions,
           Wq, bq, Wk, bk, Wv, bv, Wo, bo, _time_iters=0):
    use_mask = not bool(np.all(np.asarray(mask) > 0))
    nc, run = _get(use_mask)
    in_maps = make_in_maps(inputs_q, inputs_kv, mask, q_positions,
                           kv_positions, Wq, bq, Wk, bk, Wv, bv, Wo, bo,
                           use_mask)
    results, times = run(in_maps, time_iters=_time_iters)
    acc = np.zeros((D, T), np.float64)
    for c in range(NCORES):
        acc += results[c]["outT"].astype(np.float64)
    bo_full = (np.asarray(Wo, np.float64) @ np.asarray(bv, np.float64)
               + np.asarray(bo, np.float64))
    acc += bo_full[:, None]
    out = acc.astype(np.float32).reshape(D, B, TQ).transpose(2, 1, 0)
    out = np.ascontiguousarray(out)
    if _time_iters:
        kernel._last_times = times
    return out


# revision 85
# speedup vs baseline: 5.3503x; 1.0389x over previous
"""CrossAttention Trainium2 kernel — 8-core tensor-parallel (2 heads/core).

Self-contained: builds a Bass/Tile kernel, shards the full inputs across the
8 NeuronCores, runs via the axon PJRT path, and gathers the full output.

Per-core layout (core c owns heads 2c, 2c+1 = 128 of 1024 model dims):
  phase P: q/k/v projections (fp16 matmuls, fp32 PSUM) + rotary applied with
           host-precomputed cos/sin tables; the "spliced" operand comes from a
           partition-pair-swapped SBUF->SBUF DMA copy (sign folded into sin).
  phase A: per (b, qb, h): QK^T scores into 2-bank PSUM tiles, batched Exp on
           the Act engine, PV with q-tokens on PSUM partitions (out free = 65:
           64 channels + a ones-column giving the softmax denominator), then
           normalize on evacuation via a per-partition reciprocal scale.
  phase O: transpose attention output back to [chan, tok] via identity
           matmuls, single-pass output projection, partial fp16 outputs summed
           (+ Wo@bv + bo folded in) on the host.
"""

import sys
import time

for _p in ("/opt/trn_rl_repo", "/root/.axon_site/_ro/trn_rl_repo"):
    if _p not in sys.path:
        sys.path.insert(0, _p)

import numpy as np
from contextlib import ExitStack

import concourse.bacc as bacc
import concourse.mybir as mybir
import concourse.tile as tile
from concourse.mybir import ActivationFunctionType as AF
from concourse.mybir import AluOpType as ALU

# ---------------------------------------------------------------- problem dims
D = 1024
H = 16
DH = 64
TQ = 2048
TKV = 2048
B = 2
NCORES = 8
HPC = H // NCORES          # heads per core = 2
MPC = HPC * DH             # dims per core  = 128
T = B * TQ                 # token axis (b-grouped) = 4096
NROT = 32                  # rotated channels per head (frac 0.5 of 64)
NHEADS_ROT = 12            # rotated heads (frac 0.75 of 16)
MAX_WL = 8192.0

F32 = mybir.dt.float32
F16 = mybir.dt.float16
NPF16 = np.float16

KC = D // 128              # 8 contraction chunks for projections
TB = T // 512              # 8 token blocks of 512
TBB = TQ // 512            # 4 token blocks per batch
QB = TQ // 512             # 4 query blocks per batch
KVC = TKV // 128           # 16 kv chunks per batch
VW = 2 * (DH + 1)          # vn chunk width: [h0 ch, ones, h1 ch, ones] = 130


# ---------------------------------------------------------------- bass builder
def build_kernel(use_mask: bool):
    nc = bacc.Bacc("TRN2", target_bir_lowering=False, debug=False,
                   enable_asserts=True, num_devices=NCORES)

    xqT = nc.dram_tensor("xqT", [D, T], F16, kind="ExternalInput").ap()
    xkvT = nc.dram_tensor("xkvT", [D, T], F16, kind="ExternalInput").ap()
    # w3 packs [wkT | wvT | wqT] so the whole projection weight set is one
    # DMA; bqk packs [bk | bq]; tabq/tabk pack [cos_b0|sin_b0|cos_b1|sin_b1]
    # so each batch-half of a rotary table is one DMA. Fewer startup DMAs
    # matter: issue overhead is ~1.25us per DMA on an in-order queue.
    w3_d = nc.dram_tensor("w3", [D, 3 * MPC], F16, kind="ExternalInput").ap()
    bqk_d = nc.dram_tensor("bqk", [MPC, 2], F32, kind="ExternalInput").ap()
    woT_d = nc.dram_tensor("woT", [MPC, D], F16, kind="ExternalInput").ap()
    tabq_d = nc.dram_tensor("tabq", [128, 2 * T], F16,
                            kind="ExternalInput").ap()
    tabk_d = nc.dram_tensor("tabk", [128, 2 * T], F16,
                            kind="ExternalInput").ap()
    iden_d = nc.dram_tensor("iden", [128, 128], F16, kind="ExternalInput").ap()
    if use_mask:
        mask_d = nc.dram_tensor("maskT", [TKV, B, TQ], F16,
                                kind="ExternalInput").ap()
    outT = nc.dram_tensor("outT", [D, T], F16, kind="ExternalOutput").ap()

    with tile.TileContext(nc) as tc:
        with ExitStack() as octx:
            persist = octx.enter_context(tc.tile_pool(name="persist", bufs=1))
            xio = octx.enter_context(tc.tile_pool(name="xio", bufs=2))
            rotp = octx.enter_context(tc.tile_pool(name="rotp", bufs=4))
            ptp = octx.enter_context(tc.tile_pool(name="ptp", bufs=16))
            smlp = octx.enter_context(tc.tile_pool(name="smlp", bufs=8))
            otqp = octx.enter_context(tc.tile_pool(name="otqp", bufs=16))
            osb = octx.enter_context(tc.tile_pool(name="osb", bufs=10))
            pjp = octx.enter_context(tc.tile_pool(name="pjp", bufs=2,
                                                  space="PSUM"))
            stp = octx.enter_context(tc.tile_pool(name="stp", bufs=2,
                                                  space="PSUM"))
            pvp = octx.enter_context(tc.tile_pool(name="pvp", bufs=2,
                                                  space="PSUM"))
            if use_mask:
                mkp = octx.enter_context(tc.tile_pool(name="mkp", bufs=8))

            # -------- persistent tensors
            qT = persist.tile([128, T], F16, tag="qT")
            kT = persist.tile([128, T], F16, tag="kT")
            ot_t = persist.tile([128, T], F16, tag="ot_t")
            tabq = persist.tile([128, 2 * T], F16, tag="tabq")
            tabk = persist.tile([128, 2 * T], F16, tag="tabk")
            # vn chunk layout [ones | h0 chans | h1 chans | ones] so v-evac is
            # a single [128,128] copy and both heads' PV moving APs (chans +
            # their ones column) stay contiguous.
            vn = persist.tile([128, B * KVC * VW], F16, tag="vn")
            vn3 = vn.rearrange("p (c w) -> p c w", w=VW)
            woT = persist.tile([MPC, D], F16, tag="woT")
            iden = persist.tile([128, 128], F16, tag="iden")
            bqk_sb = persist.tile([MPC, 2], F32, tag="bqk")
            w3_all = persist.tile([128, KC, 3 * MPC], F16, tag="w3_all")

            # kv-side weights + first xkv block + b0-half k tables only: the
            # SP DMA queue is kept in just-in-time order so the startup
            # critical path (kv0, kv1, q0, first QK halves) is fed first.
            # Everything else is emitted later, interleaved into the
            # schedule.
            nc.sync.dma_start(wk_all[:],
                              wkT_d.rearrange("(c p) m -> p c m", p=128))
            nc.sync.dma_start(wv_all[:],
                              wvT_d.rearrange("(c p) m -> p c m", p=128))
            nc.sync.dma_start(bk_sb[:], bk_d[:])
            xq3 = xqT.rearrange("(c p) t -> p c t", p=128)
            xkv3 = xkvT.rearrange("(c p) t -> p c t", p=128)
            pre0kv = xio.tile([128, KC, 512], F16, tag="xkv", name="xkv_t0",
                                  bufs=3)
            nc.sync.dma_start(pre0kv[:], xkv3[:, :, 0:512])
            nc.sync.dma_start(cosk[:, 0:TQ], cosk_d[:, 0:TQ])
            nc.sync.dma_start(sink[:, 0:TQ], sink_d[:, 0:TQ])
            pre1kv = xio.tile([128, KC, 512], F16, tag="xkv", name="xkv_t1",
                                  bufs=3)
            nc.sync.dma_start(pre1kv[:], xkv3[:, :, 512:1024])
            nc.sync.dma_start(wq_all[:],
                              wqT_d.rearrange("(c p) m -> p c m", p=128))
            nc.sync.dma_start(bq_sb[:], bq_d[:])
            pre0q = xio.tile([128, KC, 512], F16, tag="xq", name="xq_t0")
            nc.sync.dma_start(pre0q[:], xq3[:, :, 0:512])
            nc.sync.dma_start(cosq[:, 0:TQ], cosq_d[:, 0:TQ])
            nc.sync.dma_start(sinq[:, 0:TQ], sinq_d[:, 0:TQ])
            pre2kv = xio.tile([128, KC, 512], F16, tag="xkv", name="xkv_t2",
                              bufs=3)
            nc.sync.dma_start(pre2kv[:], xkv3[:, :, 1024:1536])
            nc.vector.memset(vn3[:, :, 0:1], 1.0)
            nc.vector.memset(vn3[:, :, 2 * DH + 1:2 * DH + 2], 1.0)
            wq_sb = [wq_all[:, kc, :] for kc in range(KC)]
            wk_sb = [wk_all[:, kc, :] for kc in range(KC)]
            wv_sb = [wv_all[:, kc, :] for kc in range(KC)]

            # -------- emit helpers ------------------------------------------
            def _rotary(xt, cs, sn, sl, dma=None):
                """x = x*cos + pairswap(x)*sin; swap via SBUF->SBUF DMA,
                splice sign folded into the sin table. Startup blocks issue
                the swap from Act (idle until attention starts) so the
                compute-dependent copies don't head-of-line block SP's
                input-load queue."""
                dma = dma or nc.sync
                sh = rotp.tile([128, 512], F16, tag="sh", name="sh")
                dma.dma_start(sh[0:127:2, :], xt[1:128:2, sl])
                dma.dma_start(sh[1:128:2, :], xt[0:127:2, sl])
                nc.vector.tensor_mul(sh[:], sh[:], sn[:, sl])
                nc.vector.tensor_mul(xt[:, sl], xt[:, sl], cs[:, sl])
                nc.vector.tensor_add(xt[:, sl], xt[:, sl], sh[:])

            xkv_tiles = {}

            def _xkv(tb, pre=None):
                if tb not in xkv_tiles:
                    if pre is not None:
                        xkv_tiles[tb] = pre
                    else:
                        t = xio.tile([128, KC, 512], F16, tag="xkv",
                                     name="xkv_t", bufs=3)
                        nc.sync.dma_start(t[:],
                                          xkv3[:, :, tb * 512:(tb + 1) * 512])
                        xkv_tiles[tb] = t
                return xkv_tiles[tb]

            def kvk_part(tb, pre=None, dma=None):
                """k projection + k rotary for one 512-token block."""
                sl = slice(tb * 512, (tb + 1) * 512)
                xkv_t = _xkv(tb, pre)
                k_ps = pjp.tile([128, 512], F32, tag="pj", name="k_ps")
                for kc in range(KC):
                    nc.tensor.matmul(k_ps[:], wk_sb[kc], xkv_t[:, kc, :],
                                     start=(kc == 0), stop=(kc == KC - 1))
                nc.vector.tensor_scalar(kT[:, sl], k_ps[:], bk_sb[:], None,
                                        ALU.add)
                _rotary(kT, cosk, sink, sl, dma)

            def kvv_part(tb):
                """v projection for one block; natural [tok, chan] layout."""
                xkv_t = _xkv(tb)
                v_ps = pjp.tile([128, 512], F32, tag="pj", name="v_ps")
                for tc4 in range(4):
                    vsl = slice(tc4 * 128, (tc4 + 1) * 128)
                    for kc in range(KC):
                        nc.tensor.matmul(v_ps[:, vsl], xkv_t[:, kc, vsl],
                                         wv_sb[kc], start=(kc == 0),
                                         stop=(kc == KC - 1))
                for tc4 in range(4):
                    g = tb * 4 + tc4
                    nc.vector.tensor_copy(vn3[:, g, 1:2 * DH + 1],
                                          v_ps[:, tc4 * 128:(tc4 + 1) * 128])
                del xkv_tiles[tb]

            def kv_part(tb, pre=None, dma=None):
                kvk_part(tb, pre, dma)
                kvv_part(tb)

            def q_part(tb, pre=None, dma=None):
                """q projection + rotary for one 512-token block."""
                sl = slice(tb * 512, (tb + 1) * 512)
                if pre is None:
                    xq_t = xio.tile([128, KC, 512], F16, tag="xq",
                                    name="xq_t")
                    nc.sync.dma_start(xq_t[:], xq3[:, :, sl])
                else:
                    xq_t = pre
                q_ps = pjp.tile([128, 512], F32, tag="pj", name="q_ps")
                for kc in range(KC):
                    nc.tensor.matmul(q_ps[:], wq_sb[kc], xq_t[:, kc, :],
                                     start=(kc == 0), stop=(kc == KC - 1))
                nc.vector.tensor_scalar(qT[:, sl], q_ps[:], bq_sb[:], None,
                                        ALU.add)
                _rotary(qT, cosq, sinq, sl, dma)

            otq_tiles = {}

            def qk_exp(b, qb, h, halves=None, pts=None):
                """scores + exp for one (batch, 512-query-block, head)."""
                hsl = slice(h * DH, (h + 1) * DH)
                qsl = slice(b * TQ + qb * 512, b * TQ + (qb + 1) * 512)
                mts = mask_tiles.get((b, qb)) if use_mask else None
                if pts is None:
                    pts = []
                for half in (range(KVC // 2) if halves is None else halves):
                    st = stp.tile([128, 1024], F32, tag="st", name="st")
                    for j in range(2):
                        kc = half * 2 + j
                        nc.tensor.matmul(
                            st[:, j * 512:(j + 1) * 512],
                            kT[hsl, b * TKV + kc * 128:b * TKV + (kc + 1) * 128],
                            qT[hsl, qsl], start=True, stop=True)
                    pt = ptp.tile([128, 1024], F16, tag="pt", name="pt")
                    nc.scalar.activation(pt[:], st[:], AF.Exp)
                    if use_mask:
                        nc.vector.tensor_mul(pt[:], pt[:], mts[half][:])
                    pts.append(pt)
                return pts

            def pv_norm(b, qb, h, pts):
                """PV with q on partitions, denominator col, normalize+evac."""
                ov = pvp.tile([128, 4 * (DH + 1)], F32, tag="pv", name="ov")
                for qt in range(4):
                    osl = slice(qt * (DH + 1), (qt + 1) * (DH + 1))
                    for kc in range(KVC):
                        pcol = (kc % 2) * 512 + qt * 128
                        nc.tensor.matmul(
                            ov[:, osl],
                            pts[kc // 2][:, pcol:pcol + 128],
                            vn3[:, b * KVC + kc, h * (DH + 1):
                                (h + 1) * (DH + 1)],
                            start=(kc == 0), stop=(kc == KVC - 1))
                s_off = 0 if h == 0 else DH       # ones col position per head
                c_off = 1 if h == 0 else 0
                for qt in range(4):
                    base = qt * (DH + 1)
                    rec = smlp.tile([128, 1], F32, tag="rec", name="rec")
                    nc.vector.reciprocal(
                        rec[:], ov[:, base + s_off:base + s_off + 1])
                    otq = otqp.tile([128, DH], F16, tag="otq", name="otq")
                    nc.vector.tensor_scalar(
                        otq[:], ov[:, base + c_off:base + c_off + DH],
                        rec[:], None, ALU.mult)
                    otq_tiles[(h, qt)] = otq

            def transposes(b, qb):
                """[q, chan] -> ot_t[chan, tok] via identity matmuls."""
                for qt in range(4):
                    tr = pvp.tile([128, 4 * (DH + 1)], F32, tag="pv",
                                  name="tr")
                    for h in range(HPC):
                        nc.tensor.matmul(tr[h * DH:(h + 1) * DH, 0:128],
                                         otq_tiles[(h, qt)][:], iden[:],
                                         start=True, stop=True)
                    col = b * TQ + qb * 512 + qt * 128
                    nc.vector.tensor_copy(ot_t[:, col:col + 128],
                                          tr[:, 0:128])

            osb_tiles = {}

            def outproj_quarter(b, jc, tb4, evac_act=False):
                """one [128,512] token-quarter of output row-block jc."""
                key = (b, jc)
                if key not in osb_tiles:
                    osb_tiles[key] = osb.tile([128, 2048], F16, tag="o_sb",
                                              name="o_sb")
                o_big = osb_tiles[key]
                o_ps = pjp.tile([128, 512], F32, tag="pj", name="o_ps")
                col = b * TQ + tb4 * 512
                nc.tensor.matmul(o_ps[:], woT[:, jc * 128:(jc + 1) * 128],
                                 ot_t[:, col:col + 512], start=True, stop=True)
                c0 = tb4 * 512
                if evac_act:     # tail quarters: Act is idle after last exp
                    nc.scalar.activation(o_big[:, c0:c0 + 512], o_ps[:],
                                         AF.Identity)
                else:
                    nc.vector.tensor_copy(o_big[:, c0:c0 + 512], o_ps[:])
                nc.gpsimd.dma_start(
                    outT[jc * 128:(jc + 1) * 128,
                         b * TQ + c0:b * TQ + c0 + 512],
                    o_big[:, c0:c0 + 512])

            mask_tiles = {}

            def load_mask(b, qb):
                if not use_mask or (b, qb) in mask_tiles:
                    return
                mts = []
                for half in range(KVC // 2):
                    mt = mkp.tile([128, 1024], F16, tag="mk", name="mt")
                    for j in range(2):
                        kc = half * 2 + j
                        nc.sync.dma_start(
                            mt[:, j * 512:(j + 1) * 512],
                            mask_d[kc * 128:(kc + 1) * 128, b,
                                   qb * 512:(qb + 1) * 512])
                    mts.append(mt)
                mask_tiles[(b, qb)] = mts

            # -------- schedule ----------------------------------------------
            # kv-side of b0 projects first and the first combo's QK halves
            # interleave with the remaining kv blocks, so exp starts as soon
            # as kv blocks 0-1 + q block 0 are rotated. The remaining q/kv
            # blocks and the output projection are interleaved into the
            # attention pipeline as PE fillers inside Act-bound exp windows.
            # One-combo-deep software pipeline: QK+exp of combo i issues
            # before PV of combo i-1. DMAs are emitted just-in-time so SP's
            # in-order queue feeds the startup critical path first.
            kv_part(0, pre=pre0kv)
            kv_part(1, pre=pre1kv)
            q_part(0, pre=pre0q)
            load_mask(0, 0)
            kv_part(2, pre=pre2kv)
            pts0 = qk_exp(0, 0, 0, halves=[0, 1, 2, 3])
            nc.sync.dma_start(iden[:], iden_d[:])
            qk_exp(0, 0, 0, halves=[4, 5], pts=pts0)
            kv_part(3)
            nc.sync.dma_start(cosk[:, TQ:T], cosk_d[:, TQ:T])
            nc.sync.dma_start(sink[:, TQ:T], sink_d[:, TQ:T])
            qk_exp(0, 0, 0, halves=[6, 7], pts=pts0)
            nc.sync.dma_start(cosq[:, TQ:T], cosq_d[:, TQ:T])
            nc.sync.dma_start(sinq[:, TQ:T], sinq_d[:, TQ:T])
            nc.sync.dma_start(woT[:], woT_d[:])

            combos = [(b, qb, h) for b in range(B) for qb in range(QB)
                      for h in range(HPC)]
            fillers = {
                (0, 0, 1): [("q", 1)],
                (0, 1, 0): [("kv", 4)], (0, 1, 1): [("q", 2)],
                (0, 2, 0): [("kv", 5)], (0, 2, 1): [("q", 3), ("q", 4)],
                (0, 3, 0): [("kv", 6)], (0, 3, 1): [("kv", 7)],
                (1, 0, 0): [("q", 5)],
                (1, 1, 0): [("q", 6)],
                (1, 2, 0): [("q", 7)],
            }
            # outproj quarters: (0,*) available from combo index 9 (after
            # transposes(0,3) at index 8); (1,*,tb4) needs transposes(1,tb4),
            # emitted during combo index 10+2*tb4.
            opq = {9: [(0, 0), (0, 1)], 10: [(0, 2), (0, 3)],
                   11: [(0, 4), (0, 5)], 12: [(0, 6), (0, 7)]}
            for i, jcs in opq.items():
                fillers.setdefault(combos[i], []).extend(
                    ("op", b, jc, tb4) for b, jc in jcs for tb4 in range(4))
            h0slots = {13: [0, 1, 2], 14: [3, 4, 5], 15: [6, 7]}
            for i, jcs in h0slots.items():
                fillers.setdefault(combos[i], []).extend(
                    ("op", 1, jc, tb4) for jc in jcs for tb4 in (0, 1))
            # t2 quarters fit in the final exp window (transposes(1,2) are
            # emitted during combo 14)
            fillers.setdefault(combos[15], []).extend(
                ("op", 1, jc, 2) for jc in range(8))
            prev = (0, 0, 0)
            pts_prev = pts0
            for c in combos[1:]:
                load_mask(c[0], c[1])
                pts = qk_exp(*c)
                for f in fillers.get(c, []):
                    if f[0] == "q":
                        q_part(f[1])
                    elif f[0] == "kv":
                        kv_part(f[1])
                    else:
                        outproj_quarter(f[1], f[2], f[3])
                if prev is not None:
                    pv_norm(*prev, pts_prev)
                    if prev[2] == HPC - 1:
                        transposes(prev[0], prev[1])
                prev, pts_prev = c, pts
            pv_norm(*prev, pts_prev)
            transposes(prev[0], prev[1])
            for jc in range(8):
                outproj_quarter(1, jc, 3, evac_act=True)

    nc.compile()
    return nc


# ---------------------------------------------------------------- pjrt runner
def _make_runner(nc, n_cores=NCORES):
    import jax
    from jax.sharding import Mesh, PartitionSpec
    from jax.experimental.shard_map import shard_map
    from concourse.bass2jax import (_bass_exec_p, install_neuronx_cc_hook,
                                    partition_id_tensor)

    install_neuronx_cc_hook()
    partition_name = (nc.partition_id_tensor.name
                      if nc.partition_id_tensor else None)
    in_names, out_names, out_avals, zero_shapes = [], [], [], []
    for alloc in nc.m.functions[0].allocations:
        if not isinstance(alloc, mybir.MemoryLocationSet):
            continue
        name = alloc.memorylocations[0].name
        if alloc.kind == "ExternalInput":
            if name != partition_name:
                in_names.append(name)
        elif alloc.kind == "ExternalOutput":
            shape = tuple(alloc.tensor_shape)
            dtype = mybir.dt.np(alloc.dtype)
            out_names.append(name)
            out_avals.append(jax.core.ShapedArray(shape, dtype))
            zero_shapes.append((shape, dtype))
    n_params = len(in_names)
    n_outs = len(out_avals)
    all_in_names = list(in_names) + list(out_names)
    if partition_name is not None:
        all_in_names.append(partition_name)

    def _body(*args):
        operands = list(args)
        if partition_name is not None:
            operands.append(partition_id_tensor())
        return tuple(_bass_exec_p.bind(
            *operands, out_avals=tuple(out_avals), in_names=tuple(all_in_names),
            out_names=tuple(out_names), lowering_input_output_aliases=(),
            sim_require_finite=True, sim_require_nnan=True, nc=nc))

    devices = jax.devices()[:n_cores]
    mesh = Mesh(np.asarray(devices), ("core",))
    in_specs = (PartitionSpec("core"),) * (n_params + n_outs)
    out_specs = (PartitionSpec("core"),) * len(out_names)
    donate = tuple(range(n_params, n_params + n_outs))
    sharded = jax.jit(
        shard_map(_body, mesh=mesh, in_specs=in_specs, out_specs=out_specs,
                  check_rep=False),
        donate_argnums=donate, keep_unused=True)

    def run(in_maps, time_iters=0):
        per_core = [[np.asarray(m[name]) for name in in_names]
                    for m in in_maps]
        concat_in = [np.concatenate([per_core[c][i] for c in range(n_cores)],
                                    axis=0) for i in range(n_params)]

        def zeros():
            return [np.zeros((n_cores * s[0], *s[1:]), d)
                    for s, d in zero_shapes]

        import jax
        out_arrs = sharded(*concat_in, *zeros())
        jax.block_until_ready(out_arrs)
        times = []
        for _ in range(time_iters):
            t0 = time.perf_counter()
            o = sharded(*concat_in, *zeros())
            jax.block_until_ready(o)
            times.append(time.perf_counter() - t0)
            out_arrs = o
        results = [
            {name: np.asarray(out_arrs[i]).reshape(n_cores,
                                                   *out_avals[i].shape)[c]
             for i, name in enumerate(out_names)}
            for c in range(n_cores)]
        return results, times

    return run


# ---------------------------------------------------------------- host shard
def _tables(positions, core, npos_dtype=np.float64):
    """cos/sin [128, T] fp16 tables; sign of the splice folded into sin."""
    pos = np.asarray(positions, np.float64).T.reshape(T)   # b-major tokens
    nb = NROT // 2
    freq = MAX_WL ** (2.0 / NROT * np.linspace(0.0, float(nb), nb))
    inv = 1.0 / freq                                        # [16]
    cos = np.ones((128, T), np.float64)
    sin = np.zeros((128, T), np.float64)
    for hl in range(HPC):
        hglob = core * HPC + hl
        if hglob >= NHEADS_ROT:
            continue
        for cc in range(NROT):
            ang = pos * inv[cc // 2]
            r = hl * DH + cc
            cos[r] = np.cos(ang)
            sgn = -1.0 if cc % 2 == 0 else 1.0
            sin[r] = sgn * np.sin(ang)
    return cos.astype(NPF16), sin.astype(NPF16)


def make_in_maps(inputs_q, inputs_kv, mask, q_positions, kv_positions,
                 Wq, bq, Wk, bk, Wv, bv, Wo, bo, use_mask):
    f32 = np.float32
    xqT = np.ascontiguousarray(
        np.asarray(inputs_q, f32).transpose(2, 1, 0).reshape(D, T)).astype(NPF16)
    xkvT = np.ascontiguousarray(
        np.asarray(inputs_kv, f32).transpose(2, 1, 0).reshape(D, T)).astype(NPF16)
    scale = f32(1.0 / np.sqrt(DH))
    Wq, Wk, Wv, Wo = (np.asarray(a, f32) for a in (Wq, Wk, Wv, Wo))
    bq, bk, bv, bo = (np.asarray(a, f32) for a in (bq, bk, bv, bo))
    iden = np.eye(128, dtype=NPF16)
    if use_mask:
        maskT = np.ascontiguousarray((np.asarray(mask) > 0).astype(NPF16))

    in_maps = []
    for c in range(NCORES):
        sl = slice(c * MPC, (c + 1) * MPC)
        cq, sq = _tables(q_positions, c)
        ck, sk = _tables(kv_positions, c)
        m = {
            "xqT": xqT, "xkvT": xkvT,
            "wqT": np.ascontiguousarray((scale * Wq[sl, :]).T).astype(NPF16),
            "wkT": np.ascontiguousarray(Wk[sl, :].T).astype(NPF16),
            "wvT": np.ascontiguousarray(Wv[sl, :].T).astype(NPF16),
            "bq": (scale * bq[sl]).reshape(MPC, 1),
            "bk": bk[sl].reshape(MPC, 1).copy(),
            "woT": np.ascontiguousarray(Wo[:, sl].T).astype(NPF16),
            "cosq": cq, "sinq": sq, "cosk": ck, "sink": sk,
            "iden": iden,
        }
        if use_mask:
            m["maskT"] = maskT
        in_maps.append(m)
    return in_maps


_CACHE = {}


def _get(use_mask):
    if use_mask not in _CACHE:
        nc = build_kernel(use_mask)
        _CACHE[use_mask] = (nc, _make_runner(nc))
    return _CACHE[use_mask]


def kernel(inputs_q, inputs_kv, mask, q_positions, kv_positions,
           Wq, bq, Wk, bk, Wv, bv, Wo, bo, _time_iters=0):
    use_mask = not bool(np.all(np.asarray(mask) > 0))
    nc, run = _get(use_mask)
    in_maps = make_in_maps(inputs_q, inputs_kv, mask, q_positions,
                           kv_positions, Wq, bq, Wk, bk, Wv, bv, Wo, bo,
                           use_mask)
    results, times = run(in_maps, time_iters=_time_iters)
    acc = np.zeros((D, T), np.float64)
    for c in range(NCORES):
        acc += results[c]["outT"].astype(np.float64)
    bo_full = (np.asarray(Wo, np.float64) @ np.asarray(bv, np.float64)
               + np.asarray(bo, np.float64))
    acc += bo_full[:, None]
    out = acc.astype(np.float32).reshape(D, B, TQ).transpose(2, 1, 0)
    out = np.ascontiguousarray(out)
    if _time_iters:
        kernel._last_times = times
    return out


# revision 100
# speedup vs baseline: 5.3982x; 1.0090x over previous
"""CrossAttention Trainium2 kernel — 8-core tensor-parallel (2 heads/core).

Self-contained: builds a Bass/Tile kernel, shards the full inputs across the
8 NeuronCores, runs via the axon PJRT path, and gathers the full output.

Per-core layout (core c owns heads 2c, 2c+1 = 128 of 1024 model dims):
  phase P: q/k/v projections (fp16 matmuls, fp32 PSUM) + rotary applied with
           host-precomputed cos/sin tables; the "spliced" operand comes from a
           partition-pair-swapped SBUF->SBUF DMA copy (sign folded into sin).
  phase A: per (b, qb, h): QK^T scores into 2-bank PSUM tiles, batched Exp on
           the Act engine, PV with q-tokens on PSUM partitions (out free = 65:
           64 channels + a ones-column giving the softmax denominator), then
           normalize on evacuation via a per-partition reciprocal scale.
  phase O: transpose attention output back to [chan, tok] via identity
           matmuls, single-pass output projection, partial fp16 outputs summed
           (+ Wo@bv + bo folded in) on the host.
"""

import sys
import time

for _p in ("/opt/trn_rl_repo", "/root/.axon_site/_ro/trn_rl_repo"):
    if _p not in sys.path:
        sys.path.insert(0, _p)

import numpy as np
from contextlib import ExitStack

import concourse.bacc as bacc
import concourse.mybir as mybir
import concourse.tile as tile
from concourse.mybir import ActivationFunctionType as AF
from concourse.mybir import AluOpType as ALU

# ---------------------------------------------------------------- problem dims
D = 1024
H = 16
DH = 64
TQ = 2048
TKV = 2048
B = 2
NCORES = 8
HPC = H // NCORES          # heads per core = 2
MPC = HPC * DH             # dims per core  = 128
T = B * TQ                 # token axis (b-grouped) = 4096
NROT = 32                  # rotated channels per head (frac 0.5 of 64)
NHEADS_ROT = 12            # rotated heads (frac 0.75 of 16)
MAX_WL = 8192.0

F32 = mybir.dt.float32
F16 = mybir.dt.float16
NPF16 = np.float16

KC = D // 128              # 8 contraction chunks for projections
TB = T // 512              # 8 token blocks of 512
TBB = TQ // 512            # 4 token blocks per batch
QB = TQ // 512             # 4 query blocks per batch
KVC = TKV // 128           # 16 kv chunks per batch
VW = 2 * (DH + 1)          # vn chunk width: [h0 ch, ones, h1 ch, ones] = 130


# ---------------------------------------------------------------- bass builder
def build_kernel(use_mask: bool):
    nc = bacc.Bacc("TRN2", target_bir_lowering=False, debug=False,
                   enable_asserts=True, num_devices=NCORES)

    xqT = nc.dram_tensor("xqT", [D, T], F16, kind="ExternalInput").ap()
    xkvT = nc.dram_tensor("xkvT", [D, T], F16, kind="ExternalInput").ap()
    # w3 packs [wkT | wvT | wqT] so the whole projection weight set is one
    # DMA; bqk packs [bk | bq]; tabq/tabk pack [cos_b0|sin_b0|cos_b1|sin_b1]
    # so each batch-half of a rotary table is one DMA. Fewer startup DMAs
    # matter: issue overhead is ~1.25us per DMA on an in-order queue.
    w3_d = nc.dram_tensor("w3", [D, 3 * MPC], F16, kind="ExternalInput").ap()
    bqk_d = nc.dram_tensor("bqk", [MPC, 2], F32, kind="ExternalInput").ap()
    woT_d = nc.dram_tensor("woT", [MPC, D], F16, kind="ExternalInput").ap()
    tabq_d = nc.dram_tensor("tabq", [128, 2 * T], F16,
                            kind="ExternalInput").ap()
    tabk_d = nc.dram_tensor("tabk", [128, 2 * T], F16,
                            kind="ExternalInput").ap()
    idpm_d = nc.dram_tensor("idpm", [128, 256], F16,
                            kind="ExternalInput").ap()
    if use_mask:
        mask_d = nc.dram_tensor("maskT", [TKV, B, TQ], F16,
                                kind="ExternalInput").ap()
    outT = nc.dram_tensor("outT", [D, T], F16, kind="ExternalOutput").ap()

    with tile.TileContext(nc) as tc:
        with ExitStack() as octx:
            persist = octx.enter_context(tc.tile_pool(name="persist", bufs=1))
            xio = octx.enter_context(tc.tile_pool(name="xio", bufs=2))
            rotp = octx.enter_context(tc.tile_pool(name="rotp", bufs=4))
            ptp = octx.enter_context(tc.tile_pool(name="ptp", bufs=16))
            smlp = octx.enter_context(tc.tile_pool(name="smlp", bufs=8))
            otqp = octx.enter_context(tc.tile_pool(name="otqp", bufs=16))
            osb = octx.enter_context(tc.tile_pool(name="osb", bufs=10))
            pjp = octx.enter_context(tc.tile_pool(name="pjp", bufs=2,
                                                  space="PSUM"))
            stp = octx.enter_context(tc.tile_pool(name="stp", bufs=2,
                                                  space="PSUM"))
            pvp = octx.enter_context(tc.tile_pool(name="pvp", bufs=2,
                                                  space="PSUM"))
            if use_mask:
                mkp = octx.enter_context(tc.tile_pool(name="mkp", bufs=8))

            # -------- persistent tensors
            qT = persist.tile([128, T], F16, tag="qT")
            kT = persist.tile([128, T], F16, tag="kT")
            ot_t = persist.tile([128, T], F16, tag="ot_t")
            tabq = persist.tile([128, 2 * T], F16, tag="tabq")
            tabk = persist.tile([128, 2 * T], F16, tag="tabk")
            # vn chunk layout [ones | h0 chans | h1 chans | ones] so v-evac is
            # a single [128,128] copy and both heads' PV moving APs (chans +
            # their ones column) stay contiguous.
            vn = persist.tile([128, B * KVC * VW], F16, tag="vn")
            vn3 = vn.rearrange("p (c w) -> p c w", w=VW)
            woT = persist.tile([MPC, D], F16, tag="woT")
            idpm = persist.tile([128, 256], F16, tag="idpm")
            bqk_sb = persist.tile([MPC, 2], F32, tag="bqk")
            w3_all = persist.tile([128, KC, 3 * MPC], F16, tag="w3_all")

            # Startup loads in last-needed order on SP; the tiny bias load
            # goes on Act whose queue otherwise handles only the startup
            # rotary splices. Transfer completion order ~= this issue order
            # on the shared DMA engines.
            xq3 = xqT.rearrange("(c p) t -> p c t", p=128)
            xkv3 = xkvT.rearrange("(c p) t -> p c t", p=128)
            nc.sync.dma_start(w3_all[:],
                              w3_d.rearrange("(c p) m -> p c m", p=128))
            nc.scalar.dma_start(bqk_sb[:], bqk_d[:])
            nc.scalar.dma_start(idpm[:], idpm_d[:])
            pre01kv = xio.tile([128, KC, 1024], F16, tag="xkv01",
                               name="xkv_t01", bufs=1)
            nc.sync.dma_start(pre01kv[:], xkv3[:, :, 0:1024])
            nc.sync.dma_start(tabk[:, 0:2 * TQ], tabk_d[:, 0:2 * TQ])
            pre0q = xio.tile([128, KC, 512], F16, tag="xq", name="xq_t0")
            nc.scalar.dma_start(pre0q[:], xq3[:, :, 0:512])
            nc.scalar.dma_start(tabq[:, 0:2 * TQ], tabq_d[:, 0:2 * TQ])
            pre2kv = xio.tile([128, KC, 512], F16, tag="xkv", name="xkv_t2",
                              bufs=3)
            nc.sync.dma_start(pre2kv[:], xkv3[:, :, 1024:1536])
            nc.vector.memset(vn3[:, :, 0:1], 1.0)
            nc.vector.memset(vn3[:, :, 2 * DH + 1:2 * DH + 2], 1.0)
            wk_sb = [w3_all[:, kc, 0:MPC] for kc in range(KC)]
            wv_sb = [w3_all[:, kc, MPC:2 * MPC] for kc in range(KC)]
            wq_sb = [w3_all[:, kc, 2 * MPC:3 * MPC] for kc in range(KC)]
            bk_sb = bqk_sb[:, 0:1]
            bq_sb = bqk_sb[:, 1:2]

            # -------- emit helpers ------------------------------------------
            def _tab_aps(tab, tb):
                b, off = tb // 4, (tb % 4) * 512
                cs = tab[:, b * 2 * TQ + off:b * 2 * TQ + off + 512]
                sn = tab[:, b * 2 * TQ + TQ + off:b * 2 * TQ + TQ + off + 512]
                return cs, sn

            def _rotary(xt, tab, tb, dma=None, perm=False):
                """x = x*cos + pairswap(x)*sin, splice sign folded into the
                sin table. The swap is an SBUF->SBUF DMA in steady state; on
                the startup critical path (perm=True) it is a PE permutation
                matmul instead, avoiding the serialized SP DMA queue."""
                sl = slice(tb * 512, (tb + 1) * 512)
                cs, sn = _tab_aps(tab, tb)
                sh = rotp.tile([128, 512], F16, tag="sh", name="sh")
                if perm:
                    pps = stp.tile([128, 1024], F32, tag="st", name="pps")
                    nc.tensor.matmul(pps[:, 0:512], idpm[:, 128:256],
                                     xt[:, sl], start=True, stop=True)
                    nc.vector.tensor_mul(sh[:], pps[:, 0:512], sn)
                else:
                    dma = dma or nc.sync
                    dma.dma_start(sh[0:127:2, :], xt[1:128:2, sl])
                    dma.dma_start(sh[1:128:2, :], xt[0:127:2, sl])
                    nc.vector.tensor_mul(sh[:], sh[:], sn)
                nc.vector.tensor_mul(xt[:, sl], xt[:, sl], cs)
                nc.vector.tensor_add(xt[:, sl], xt[:, sl], sh[:])

            xkv_tiles = {}

            def _xkv(tb, pre=None):
                if tb not in xkv_tiles:
                    if pre is not None:
                        xkv_tiles[tb] = pre
                    else:
                        t = xio.tile([128, KC, 512], F16, tag="xkv",
                                     name="xkv_t", bufs=3)
                        nc.sync.dma_start(t[:],
                                          xkv3[:, :, tb * 512:(tb + 1) * 512])
                        xkv_tiles[tb] = t
                return xkv_tiles[tb]

            def kvk_part(tb, pre=None, dma=None, perm=False):
                """k projection + k rotary for one 512-token block."""
                sl = slice(tb * 512, (tb + 1) * 512)
                xkv_t = _xkv(tb, pre)
                k_ps = pjp.tile([128, 512], F32, tag="pj", name="k_ps")
                for kc in range(KC):
                    nc.tensor.matmul(k_ps[:], wk_sb[kc], xkv_t[:, kc, :],
                                     start=(kc == 0), stop=(kc == KC - 1))
                nc.vector.tensor_scalar(kT[:, sl], k_ps[:], bk_sb, None,
                                        ALU.add)
                _rotary(kT, tabk, tb, dma, perm)

            def kvv_part(tb, evac_act=False):
                """v projection for one block; natural [tok, chan] layout."""
                xkv_t = _xkv(tb)
                v_ps = pjp.tile([128, 512], F32, tag="pj", name="v_ps")
                for tc4 in range(4):
                    vsl = slice(tc4 * 128, (tc4 + 1) * 128)
                    for kc in range(KC):
                        nc.tensor.matmul(v_ps[:, vsl], xkv_t[:, kc, vsl],
                                         wv_sb[kc], start=(kc == 0),
                                         stop=(kc == KC - 1))
                for tc4 in range(4):
                    g = tb * 4 + tc4
                    dst = vn3[:, g, 1:2 * DH + 1]
                    src = v_ps[:, tc4 * 128:(tc4 + 1) * 128]
                    if evac_act:
                        nc.scalar.activation(dst, src, AF.Identity)
                    else:
                        nc.vector.tensor_copy(dst, src)
                del xkv_tiles[tb]

            def kv_part(tb, pre=None, dma=None, perm=False, evac_act=False):
                kvk_part(tb, pre, dma, perm)
                kvv_part(tb, evac_act)

            def q_part(tb, pre=None, dma=None, perm=False):
                """q projection + rotary for one 512-token block."""
                sl = slice(tb * 512, (tb + 1) * 512)
                if pre is None:
                    xq_t = xio.tile([128, KC, 512], F16, tag="xq",
                                    name="xq_t")
                    nc.sync.dma_start(xq_t[:], xq3[:, :, sl])
                else:
                    xq_t = pre
                q_ps = pjp.tile([128, 512], F32, tag="pj", name="q_ps")
                for kc in range(KC):
                    nc.tensor.matmul(q_ps[:], wq_sb[kc], xq_t[:, kc, :],
                                     start=(kc == 0), stop=(kc == KC - 1))
                nc.vector.tensor_scalar(qT[:, sl], q_ps[:], bq_sb, None,
                                        ALU.add)
                _rotary(qT, tabq, tb, dma, perm)

            otq_tiles = {}

            def qk_exp(b, qb, h, halves=None, pts=None):
                """scores + exp for one (batch, 512-query-block, head)."""
                hsl = slice(h * DH, (h + 1) * DH)
                qsl = slice(b * TQ + qb * 512, b * TQ + (qb + 1) * 512)
                mts = mask_tiles.get((b, qb)) if use_mask else None
                if pts is None:
                    pts = []
                for half in (range(KVC // 2) if halves is None else halves):
                    st = stp.tile([128, 1024], F32, tag="st", name="st")
                    for j in range(2):
                        kc = half * 2 + j
                        nc.tensor.matmul(
                            st[:, j * 512:(j + 1) * 512],
                            kT[hsl, b * TKV + kc * 128:b * TKV + (kc + 1) * 128],
                            qT[hsl, qsl], start=True, stop=True)
                    pt = ptp.tile([128, 1024], F16, tag="pt", name="pt")
                    nc.scalar.activation(pt[:], st[:], AF.Exp)
                    if use_mask:
                        nc.vector.tensor_mul(pt[:], pt[:], mts[half][:])
                    pts.append(pt)
                return pts

            def pv_norm(b, qb, h, pts):
                """PV with q on partitions, denominator col, normalize+evac."""
                ov = pvp.tile([128, 4 * (DH + 1)], F32, tag="pv", name="ov")
                for qt in range(4):
                    osl = slice(qt * (DH + 1), (qt + 1) * (DH + 1))
                    for kc in range(KVC):
                        pcol = (kc % 2) * 512 + qt * 128
                        nc.tensor.matmul(
                            ov[:, osl],
                            pts[kc // 2][:, pcol:pcol + 128],
                            vn3[:, b * KVC + kc, h * (DH + 1):
                                (h + 1) * (DH + 1)],
                            start=(kc == 0), stop=(kc == KVC - 1))
                s_off = 0 if h == 0 else DH       # ones col position per head
                c_off = 1 if h == 0 else 0
                for qt in range(4):
                    base = qt * (DH + 1)
                    rec = smlp.tile([128, 1], F32, tag="rec", name="rec")
                    nc.vector.reciprocal(
                        rec[:], ov[:, base + s_off:base + s_off + 1])
                    otq = otqp.tile([128, DH], F16, tag="otq", name="otq")
                    nc.vector.tensor_scalar(
                        otq[:], ov[:, base + c_off:base + c_off + DH],
                        rec[:], None, ALU.mult)
                    otq_tiles[(h, qt)] = otq

            def transposes(b, qb):
                """[q, chan] -> ot_t[chan, tok] via identity matmuls."""
                for qt in range(4):
                    tr = pvp.tile([128, 4 * (DH + 1)], F32, tag="pv",
                                  name="tr")
                    for h in range(HPC):
                        nc.tensor.matmul(tr[h * DH:(h + 1) * DH, 0:128],
                                         otq_tiles[(h, qt)][:],
                                         idpm[:, 0:128],
                                         start=True, stop=True)
                    col = b * TQ + qb * 512 + qt * 128
                    nc.vector.tensor_copy(ot_t[:, col:col + 128],
                                          tr[:, 0:128])

            osb_tiles = {}

            def outproj_quarter(b, jc, tb4, evac_act=False):
                """one [128,512] token-quarter of output row-block jc."""
                key = (b, jc)
                if key not in osb_tiles:
                    osb_tiles[key] = osb.tile([128, 2048], F16, tag="o_sb",
                                              name="o_sb")
                o_big = osb_tiles[key]
                o_ps = pjp.tile([128, 512], F32, tag="pj", name="o_ps")
                col = b * TQ + tb4 * 512
                nc.tensor.matmul(o_ps[:], woT[:, jc * 128:(jc + 1) * 128],
                                 ot_t[:, col:col + 512], start=True, stop=True)
                c0 = tb4 * 512
                if evac_act:     # tail quarters: Act is idle after last exp
                    nc.scalar.activation(o_big[:, c0:c0 + 512], o_ps[:],
                                         AF.Identity)
                else:
                    nc.vector.tensor_copy(o_big[:, c0:c0 + 512], o_ps[:])
                nc.gpsimd.dma_start(
                    outT[jc * 128:(jc + 1) * 128,
                         b * TQ + c0:b * TQ + c0 + 512],
                    o_big[:, c0:c0 + 512])

            mask_tiles = {}

            def load_mask(b, qb):
                if not use_mask or (b, qb) in mask_tiles:
                    return
                mts = []
                for half in range(KVC // 2):
                    mt = mkp.tile([128, 1024], F16, tag="mk", name="mt")
                    for j in range(2):
                        kc = half * 2 + j
                        nc.sync.dma_start(
                            mt[:, j * 512:(j + 1) * 512],
                            mask_d[kc * 128:(kc + 1) * 128, b,
                                   qb * 512:(qb + 1) * 512])
                    mts.append(mt)
                mask_tiles[(b, qb)] = mts

            # -------- schedule ----------------------------------------------
            # kv-side of b0 projects first and the first combo's QK halves
            # interleave with the remaining kv blocks, so exp starts as soon
            # as kv blocks 0-1 + q block 0 are rotated. The remaining q/kv
            # blocks and the output projection are interleaved into the
            # attention pipeline as PE fillers inside Act-bound exp windows.
            # One-combo-deep software pipeline: QK+exp of combo i issues
            # before PV of combo i-1. DMAs are emitted just-in-time so SP's
            # in-order queue feeds the startup critical path first.
            # critical path to the first exp: kvk0, q0, kvk1 projections +
            # rotaries (splices on Act's queue), QK halves as kv blocks land;
            # v-projections are deferred into the early exp windows.
            kv_part(0, pre=pre01kv[:, :, 0:512], perm=True, evac_act=True)
            kv_part(1, pre=pre01kv[:, :, 512:1024], perm=True,
                    evac_act=True)
            q_part(0, pre=pre0q, perm=True)
            load_mask(0, 0)
            kv_part(2, pre=pre2kv, perm=True, evac_act=True)
            pts0 = qk_exp(0, 0, 0, halves=[0, 1, 2, 3])
            qk_exp(0, 0, 0, halves=[4, 5], pts=pts0)
            kv_part(3, perm=True)
            nc.sync.dma_start(tabk[:, 2 * TQ:4 * TQ], tabk_d[:, 2 * TQ:4 * TQ])
            qk_exp(0, 0, 0, halves=[6, 7], pts=pts0)
            nc.sync.dma_start(tabq[:, 2 * TQ:4 * TQ], tabq_d[:, 2 * TQ:4 * TQ])
            nc.sync.dma_start(woT[:], woT_d[:])

            combos = [(b, qb, h) for b in range(B) for qb in range(QB)
                      for h in range(HPC)]
            fillers = {
                (0, 0, 1): [("q", 1)],
                (0, 1, 0): [("kv", 4)], (0, 1, 1): [("q", 2)],
                (0, 2, 0): [("kv", 5)], (0, 2, 1): [("q", 3), ("q", 4)],
                (0, 3, 0): [("kv", 6)], (0, 3, 1): [("kv", 7)],
                (1, 0, 0): [("q", 5)],
                (1, 1, 0): [("q", 6)],
                (1, 2, 0): [("q", 7)],
            }
            # outproj quarters: (0,*) available from combo index 9 (after
            # transposes(0,3) at index 8); (1,*,tb4) needs transposes(1,tb4),
            # emitted during combo index 10+2*tb4.
            opq = {9: [(0, 0), (0, 1)], 10: [(0, 2), (0, 3)],
                   11: [(0, 4), (0, 5)], 12: [(0, 6), (0, 7)]}
            for i, jcs in opq.items():
                fillers.setdefault(combos[i], []).extend(
                    ("op", b, jc, tb4) for b, jc in jcs for tb4 in range(4))
            h0slots = {13: [0, 1, 2], 14: [3, 4, 5], 15: [6, 7]}
            for i, jcs in h0slots.items():
                fillers.setdefault(combos[i], []).extend(
                    ("op", 1, jc, tb4) for jc in jcs for tb4 in (0, 1))
            # t2 quarters fit in the final exp window (transposes(1,2) are
            # emitted during combo 14)
            fillers.setdefault(combos[15], []).extend(
                ("op", 1, jc, 2) for jc in range(8))
            prev = (0, 0, 0)
            pts_prev = pts0
            for c in combos[1:]:
                load_mask(c[0], c[1])
                pts = qk_exp(*c)
                for f in fillers.get(c, []):
                    if f[0] == "q":
                        q_part(f[1])
                    elif f[0] == "kv":
                        kv_part(f[1])
                    else:
                        outproj_quarter(f[1], f[2], f[3])
                if prev is not None:
                    pv_norm(*prev, pts_prev)
                    if prev[2] == HPC - 1:
                        transposes(prev[0], prev[1])
                prev, pts_prev = c, pts
            pv_norm(*prev, pts_prev)
            transposes(prev[0], prev[1])
            for jc in range(8):
                outproj_quarter(1, jc, 3, evac_act=True)

    nc.compile()
    return nc


# ---------------------------------------------------------------- pjrt runner
def _make_runner(nc, n_cores=NCORES):
    import jax
    from jax.sharding import Mesh, PartitionSpec
    from jax.experimental.shard_map import shard_map
    from concourse.bass2jax import (_bass_exec_p, install_neuronx_cc_hook,
                                    partition_id_tensor)

    install_neuronx_cc_hook()
    partition_name = (nc.partition_id_tensor.name
                      if nc.partition_id_tensor else None)
    in_names, out_names, out_avals, zero_shapes = [], [], [], []
    for alloc in nc.m.functions[0].allocations:
        if not isinstance(alloc, mybir.MemoryLocationSet):
            continue
        name = alloc.memorylocations[0].name
        if alloc.kind == "ExternalInput":
            if name != partition_name:
                in_names.append(name)
        elif alloc.kind == "ExternalOutput":
            shape = tuple(alloc.tensor_shape)
            dtype = mybir.dt.np(alloc.dtype)
            out_names.append(name)
            out_avals.append(jax.core.ShapedArray(shape, dtype))
            zero_shapes.append((shape, dtype))
    n_params = len(in_names)
    n_outs = len(out_avals)
    all_in_names = list(in_names) + list(out_names)
    if partition_name is not None:
        all_in_names.append(partition_name)

    def _body(*args):
        operands = list(args)
        if partition_name is not None:
            operands.append(partition_id_tensor())
        return tuple(_bass_exec_p.bind(
            *operands, out_avals=tuple(out_avals), in_names=tuple(all_in_names),
            out_names=tuple(out_names), lowering_input_output_aliases=(),
            sim_require_finite=True, sim_require_nnan=True, nc=nc))

    devices = jax.devices()[:n_cores]
    mesh = Mesh(np.asarray(devices), ("core",))
    in_specs = (PartitionSpec("core"),) * (n_params + n_outs)
    out_specs = (PartitionSpec("core"),) * len(out_names)
    donate = tuple(range(n_params, n_params + n_outs))
    sharded = jax.jit(
        shard_map(_body, mesh=mesh, in_specs=in_specs, out_specs=out_specs,
                  check_rep=False),
        donate_argnums=donate, keep_unused=True)

    def run(in_maps, time_iters=0):
        per_core = [[np.asarray(m[name]) for name in in_names]
                    for m in in_maps]
        concat_in = [np.concatenate([per_core[c][i] for c in range(n_cores)],
                                    axis=0) for i in range(n_params)]

        def zeros():
            return [np.zeros((n_cores * s[0], *s[1:]), d)
                    for s, d in zero_shapes]

        import jax
        out_arrs = sharded(*concat_in, *zeros())
        jax.block_until_ready(out_arrs)
        times = []
        for _ in range(time_iters):
            t0 = time.perf_counter()
            o = sharded(*concat_in, *zeros())
            jax.block_until_ready(o)
            times.append(time.perf_counter() - t0)
            out_arrs = o
        results = [
            {name: np.asarray(out_arrs[i]).reshape(n_cores,
                                                   *out_avals[i].shape)[c]
             for i, name in enumerate(out_names)}
            for c in range(n_cores)]
        return results, times

    return run


# ---------------------------------------------------------------- host shard
def _tables(positions, core, npos_dtype=np.float64):
    """cos/sin [128, T] fp16 tables; sign of the splice folded into sin."""
    pos = np.asarray(positions, np.float64).T.reshape(T)   # b-major tokens
    nb = NROT // 2
    freq = MAX_WL ** (2.0 / NROT * np.linspace(0.0, float(nb), nb))
    inv = 1.0 / freq                                        # [16]
    cos = np.ones((128, T), np.float64)
    sin = np.zeros((128, T), np.float64)
    for hl in range(HPC):
        hglob = core * HPC + hl
        if hglob >= NHEADS_ROT:
            continue
        for cc in range(NROT):
            ang = pos * inv[cc // 2]
            r = hl * DH + cc
            cos[r] = np.cos(ang)
            sgn = -1.0 if cc % 2 == 0 else 1.0
            sin[r] = sgn * np.sin(ang)
    return cos.astype(NPF16), sin.astype(NPF16)


def make_in_maps(inputs_q, inputs_kv, mask, q_positions, kv_positions,
                 Wq, bq, Wk, bk, Wv, bv, Wo, bo, use_mask):
    f32 = np.float32
    xqT = np.ascontiguousarray(
        np.asarray(inputs_q, f32).transpose(2, 1, 0).reshape(D, T)).astype(NPF16)
    xkvT = np.ascontiguousarray(
        np.asarray(inputs_kv, f32).transpose(2, 1, 0).reshape(D, T)).astype(NPF16)
    scale = f32(1.0 / np.sqrt(DH))
    Wq, Wk, Wv, Wo = (np.asarray(a, f32) for a in (Wq, Wk, Wv, Wo))
    bq, bk, bv, bo = (np.asarray(a, f32) for a in (bq, bk, bv, bo))
    iden = np.eye(128, dtype=NPF16)
    perm = np.zeros((128, 128), NPF16)
    perm[np.arange(128), np.arange(128) ^ 1] = 1.0
    idpm = np.concatenate([iden, perm], axis=1)
    if use_mask:
        maskT = np.ascontiguousarray((np.asarray(mask) > 0).astype(NPF16))

    in_maps = []
    for c in range(NCORES):
        sl = slice(c * MPC, (c + 1) * MPC)
        cq, sq = _tables(q_positions, c)
        ck, sk = _tables(kv_positions, c)
        w3 = np.concatenate(
            [Wk[sl, :].T, Wv[sl, :].T, (scale * Wq[sl, :]).T],
            axis=1)
        bqk = np.stack([bk[sl], scale * bq[sl]], axis=1)
        m = {
            "xqT": xqT, "xkvT": xkvT,
            "w3": np.ascontiguousarray(w3).astype(NPF16),
            "bqk": np.ascontiguousarray(bqk, np.float32),
            "woT": np.ascontiguousarray(Wo[:, sl].T).astype(NPF16),
            "tabq": np.ascontiguousarray(np.concatenate(
                [cq[:, :TQ], sq[:, :TQ], cq[:, TQ:], sq[:, TQ:]], axis=1)),
            "tabk": np.ascontiguousarray(np.concatenate(
                [ck[:, :TQ], sk[:, :TQ], ck[:, TQ:], sk[:, TQ:]], axis=1)),
            "idpm": idpm,
        }
        if use_mask:
            m["maskT"] = maskT
        in_maps.append(m)
    return in_maps


_CACHE = {}


def _get(use_mask):
    if use_mask not in _CACHE:
        nc = build_kernel(use_mask)
        _CACHE[use_mask] = (nc, _make_runner(nc))
    return _CACHE[use_mask]


def kernel(inputs_q, inputs_kv, mask, q_positions, kv_positions,
           Wq, bq, Wk, bk, Wv, bv, Wo, bo, _time_iters=0):
    use_mask = not bool(np.all(np.asarray(mask) > 0))
    nc, run = _get(use_mask)
    in_maps = make_in_maps(inputs_q, inputs_kv, mask, q_positions,
                           kv_positions, Wq, bq, Wk, bk, Wv, bv, Wo, bo,
                           use_mask)
    results, times = run(in_maps, time_iters=_time_iters)
    acc = np.zeros((D, T), np.float64)
    for c in range(NCORES):
        acc += results[c]["outT"].astype(np.float64)
    bo_full = (np.asarray(Wo, np.float64) @ np.asarray(bv, np.float64)
               + np.asarray(bo, np.float64))
    acc += bo_full[:, None]
    out = acc.astype(np.float32).reshape(D, B, TQ).transpose(2, 1, 0)
    out = np.ascontiguousarray(out)
    if _time_iters:
        kernel._last_times = times
    return out


# revision 104
# speedup vs baseline: 5.4044x; 1.0011x over previous
"""CrossAttention Trainium2 kernel — 8-core tensor-parallel (2 heads/core).

Self-contained: builds a Bass/Tile kernel, shards the full inputs across the
8 NeuronCores, runs via the axon PJRT path, and gathers the full output.

Per-core layout (core c owns heads 2c, 2c+1 = 128 of 1024 model dims):
  phase P: q/k/v projections (fp16 matmuls, fp32 PSUM) + rotary applied with
           host-precomputed cos/sin tables; the "spliced" operand comes from a
           partition-pair-swapped SBUF->SBUF DMA copy (sign folded into sin).
  phase A: per (b, qb, h): QK^T scores into 2-bank PSUM tiles, batched Exp on
           the Act engine, PV with q-tokens on PSUM partitions (out free = 65:
           64 channels + a ones-column giving the softmax denominator), then
           normalize on evacuation via a per-partition reciprocal scale.
  phase O: transpose attention output back to [chan, tok] via identity
           matmuls, single-pass output projection, partial fp16 outputs summed
           (+ Wo@bv + bo folded in) on the host.
"""

import sys
import time

for _p in ("/opt/trn_rl_repo", "/root/.axon_site/_ro/trn_rl_repo"):
    if _p not in sys.path:
        sys.path.insert(0, _p)

import numpy as np
from contextlib import ExitStack

import concourse.bacc as bacc
import concourse.mybir as mybir
import concourse.tile as tile
from concourse.mybir import ActivationFunctionType as AF
from concourse.mybir import AluOpType as ALU

# ---------------------------------------------------------------- problem dims
D = 1024
H = 16
DH = 64
TQ = 2048
TKV = 2048
B = 2
NCORES = 8
HPC = H // NCORES          # heads per core = 2
MPC = HPC * DH             # dims per core  = 128
T = B * TQ                 # token axis (b-grouped) = 4096
NROT = 32                  # rotated channels per head (frac 0.5 of 64)
NHEADS_ROT = 12            # rotated heads (frac 0.75 of 16)
MAX_WL = 8192.0

F32 = mybir.dt.float32
F16 = mybir.dt.float16
NPF16 = np.float16

KC = D // 128              # 8 contraction chunks for projections
TB = T // 512              # 8 token blocks of 512
TBB = TQ // 512            # 4 token blocks per batch
QB = TQ // 512             # 4 query blocks per batch
KVC = TKV // 128           # 16 kv chunks per batch
VW = 2 * (DH + 1)          # vn chunk width: [h0 ch, ones, h1 ch, ones] = 130


# ---------------------------------------------------------------- bass builder
def build_kernel(use_mask: bool):
    nc = bacc.Bacc("TRN2", target_bir_lowering=False, debug=False,
                   enable_asserts=True, num_devices=NCORES)

    xqT = nc.dram_tensor("xqT", [D, T], F16, kind="ExternalInput").ap()
    xkvT = nc.dram_tensor("xkvT", [D, T], F16, kind="ExternalInput").ap()
    # w3 packs [wkT | wvT | wqT] so the whole projection weight set is one
    # DMA; bqk packs [bk | bq]; tabq/tabk pack [cos_b0|sin_b0|cos_b1|sin_b1]
    # so each batch-half of a rotary table is one DMA. Fewer startup DMAs
    # matter: issue overhead is ~1.25us per DMA on an in-order queue.
    w3_d = nc.dram_tensor("w3", [D, 3 * MPC], F16, kind="ExternalInput").ap()
    bqk_d = nc.dram_tensor("bqk", [MPC, 2], F32, kind="ExternalInput").ap()
    woT_d = nc.dram_tensor("woT", [MPC, D], F16, kind="ExternalInput").ap()
    tabq_d = nc.dram_tensor("tabq", [128, 2 * T], F16,
                            kind="ExternalInput").ap()
    tabk_d = nc.dram_tensor("tabk", [128, 2 * T], F16,
                            kind="ExternalInput").ap()
    idpm_d = nc.dram_tensor("idpm", [128, 256], F16,
                            kind="ExternalInput").ap()
    if use_mask:
        mask_d = nc.dram_tensor("maskT", [TKV, B, TQ], F16,
                                kind="ExternalInput").ap()
    outT = nc.dram_tensor("outT", [D, T], F16, kind="ExternalOutput").ap()

    with tile.TileContext(nc) as tc:
        with ExitStack() as octx:
            persist = octx.enter_context(tc.tile_pool(name="persist", bufs=1))
            xio = octx.enter_context(tc.tile_pool(name="xio", bufs=2))
            rotp = octx.enter_context(tc.tile_pool(name="rotp", bufs=4))
            ptp = octx.enter_context(tc.tile_pool(name="ptp", bufs=16))
            smlp = octx.enter_context(tc.tile_pool(name="smlp", bufs=8))
            otqp = octx.enter_context(tc.tile_pool(name="otqp", bufs=16))
            osb = octx.enter_context(tc.tile_pool(name="osb", bufs=9))
            pjp = octx.enter_context(tc.tile_pool(name="pjp", bufs=2,
                                                  space="PSUM"))
            stp = octx.enter_context(tc.tile_pool(name="stp", bufs=2,
                                                  space="PSUM"))
            pvp = octx.enter_context(tc.tile_pool(name="pvp", bufs=2,
                                                  space="PSUM"))
            if use_mask:
                mkp = octx.enter_context(tc.tile_pool(name="mkp", bufs=8))

            # -------- persistent tensors
            qT = persist.tile([128, T], F16, tag="qT")
            kT = persist.tile([128, T], F16, tag="kT")
            ot_t = persist.tile([128, T], F16, tag="ot_t")
            tabq = persist.tile([128, 2 * T], F16, tag="tabq")
            tabk = persist.tile([128, 2 * T], F16, tag="tabk")
            # vn chunk layout [ones | h0 chans | h1 chans | ones] so v-evac is
            # a single [128,128] copy and both heads' PV moving APs (chans +
            # their ones column) stay contiguous.
            vn = persist.tile([128, B * KVC * VW], F16, tag="vn")
            vn3 = vn.rearrange("p (c w) -> p c w", w=VW)
            woT = persist.tile([MPC, D], F16, tag="woT")
            idpm = persist.tile([128, 256], F16, tag="idpm")
            bqk_sb = persist.tile([MPC, 2], F32, tag="bqk")
            w3_all = persist.tile([128, KC, 3 * MPC], F16, tag="w3_all")

            # Startup loads in last-needed order on SP; the tiny bias load
            # goes on Act whose queue otherwise handles only the startup
            # rotary splices. Transfer completion order ~= this issue order
            # on the shared DMA engines.
            xq3 = xqT.rearrange("(c p) t -> p c t", p=128)
            xkv3 = xkvT.rearrange("(c p) t -> p c t", p=128)
            nc.sync.dma_start(w3_all[:],
                              w3_d.rearrange("(c p) m -> p c m", p=128))
            nc.scalar.dma_start(bqk_sb[:], bqk_d[:])
            nc.scalar.dma_start(idpm[:], idpm_d[:])
            pre01kv = xio.tile([128, KC, 1024], F16, tag="xkv01",
                               name="xkv_t01", bufs=1)
            nc.sync.dma_start(pre01kv[:], xkv3[:, :, 0:1024])
            nc.sync.dma_start(tabk[:, 0:2048], tabk_d[:, 0:2048])
            pre0q = xio.tile([128, KC, 512], F16, tag="xq", name="xq_t0")
            nc.scalar.dma_start(pre0q[:], xq3[:, :, 0:512])
            nc.scalar.dma_start(tabq[:, 0:1024], tabq_d[:, 0:1024])
            pre2kv = xio.tile([128, KC, 512], F16, tag="xkv", name="xkv_t2",
                              bufs=3)
            nc.sync.dma_start(pre2kv[:], xkv3[:, :, 1024:1536])
            nc.vector.memset(vn3[:, :, 0:1], 1.0)
            nc.vector.memset(vn3[:, :, 2 * DH + 1:2 * DH + 2], 1.0)
            wk_sb = [w3_all[:, kc, 0:MPC] for kc in range(KC)]
            wv_sb = [w3_all[:, kc, MPC:2 * MPC] for kc in range(KC)]
            wq_sb = [w3_all[:, kc, 2 * MPC:3 * MPC] for kc in range(KC)]
            bk_sb = bqk_sb[:, 0:1]
            bq_sb = bqk_sb[:, 1:2]

            # -------- emit helpers ------------------------------------------
            def _tab_aps(tab, tb):
                # block-interleaved layout: [cos_blk | sin_blk] per 512 tokens
                base = tb * 1024
                return (tab[:, base:base + 512],
                        tab[:, base + 512:base + 1024])

            def _rotary(xt, tab, tb, dma=None, perm=False):
                """x = x*cos + pairswap(x)*sin, splice sign folded into the
                sin table. The swap is an SBUF->SBUF DMA in steady state; on
                the startup critical path (perm=True) it is a PE permutation
                matmul instead, avoiding the serialized SP DMA queue."""
                sl = slice(tb * 512, (tb + 1) * 512)
                cs, sn = _tab_aps(tab, tb)
                sh = rotp.tile([128, 512], F16, tag="sh", name="sh")
                if perm:
                    pps = stp.tile([128, 1024], F32, tag="st", name="pps")
                    nc.tensor.matmul(pps[:, 0:512], idpm[:, 128:256],
                                     xt[:, sl], start=True, stop=True)
                    nc.vector.tensor_mul(sh[:], pps[:, 0:512], sn)
                else:
                    dma = dma or nc.sync
                    dma.dma_start(sh[0:127:2, :], xt[1:128:2, sl])
                    dma.dma_start(sh[1:128:2, :], xt[0:127:2, sl])
                    nc.vector.tensor_mul(sh[:], sh[:], sn)
                nc.vector.tensor_mul(xt[:, sl], xt[:, sl], cs)
                nc.vector.tensor_add(xt[:, sl], xt[:, sl], sh[:])

            xkv_tiles = {}

            def _xkv(tb, pre=None):
                if tb not in xkv_tiles:
                    if pre is not None:
                        xkv_tiles[tb] = pre
                    else:
                        t = xio.tile([128, KC, 512], F16, tag="xkv",
                                     name="xkv_t", bufs=3)
                        nc.sync.dma_start(t[:],
                                          xkv3[:, :, tb * 512:(tb + 1) * 512])
                        xkv_tiles[tb] = t
                return xkv_tiles[tb]

            def kvk_part(tb, pre=None, dma=None, perm=False):
                """k projection + k rotary for one 512-token block."""
                sl = slice(tb * 512, (tb + 1) * 512)
                xkv_t = _xkv(tb, pre)
                k_ps = pjp.tile([128, 512], F32, tag="pj", name="k_ps")
                for kc in range(KC):
                    nc.tensor.matmul(k_ps[:], wk_sb[kc], xkv_t[:, kc, :],
                                     start=(kc == 0), stop=(kc == KC - 1))
                nc.vector.tensor_scalar(kT[:, sl], k_ps[:], bk_sb, None,
                                        ALU.add)
                _rotary(kT, tabk, tb, dma, perm)

            def kvv_part(tb, evac_act=False):
                """v projection for one block; natural [tok, chan] layout."""
                xkv_t = _xkv(tb)
                v_ps = pjp.tile([128, 512], F32, tag="pj", name="v_ps")
                for tc4 in range(4):
                    vsl = slice(tc4 * 128, (tc4 + 1) * 128)
                    for kc in range(KC):
                        nc.tensor.matmul(v_ps[:, vsl], xkv_t[:, kc, vsl],
                                         wv_sb[kc], start=(kc == 0),
                                         stop=(kc == KC - 1))
                for tc4 in range(4):
                    g = tb * 4 + tc4
                    dst = vn3[:, g, 1:2 * DH + 1]
                    src = v_ps[:, tc4 * 128:(tc4 + 1) * 128]
                    if evac_act:
                        nc.scalar.activation(dst, src, AF.Identity)
                    else:
                        nc.vector.tensor_copy(dst, src)
                del xkv_tiles[tb]

            def kv_part(tb, pre=None, dma=None, perm=False, evac_act=False):
                kvk_part(tb, pre, dma, perm)
                kvv_part(tb, evac_act)

            def q_part(tb, pre=None, dma=None, perm=False):
                """q projection + rotary for one 512-token block."""
                sl = slice(tb * 512, (tb + 1) * 512)
                if pre is None:
                    xq_t = xio.tile([128, KC, 512], F16, tag="xq",
                                    name="xq_t")
                    nc.sync.dma_start(xq_t[:], xq3[:, :, sl])
                else:
                    xq_t = pre
                q_ps = pjp.tile([128, 512], F32, tag="pj", name="q_ps")
                for kc in range(KC):
                    nc.tensor.matmul(q_ps[:], wq_sb[kc], xq_t[:, kc, :],
                                     start=(kc == 0), stop=(kc == KC - 1))
                nc.vector.tensor_scalar(qT[:, sl], q_ps[:], bq_sb, None,
                                        ALU.add)
                _rotary(qT, tabq, tb, dma, perm)

            otq_tiles = {}

            def qk_exp(b, qb, h, halves=None, pts=None):
                """scores + exp for one (batch, 512-query-block, head)."""
                hsl = slice(h * DH, (h + 1) * DH)
                qsl = slice(b * TQ + qb * 512, b * TQ + (qb + 1) * 512)
                mts = mask_tiles.get((b, qb)) if use_mask else None
                if pts is None:
                    pts = []
                for half in (range(KVC // 2) if halves is None else halves):
                    st = stp.tile([128, 1024], F32, tag="st", name="st")
                    for j in range(2):
                        kc = half * 2 + j
                        nc.tensor.matmul(
                            st[:, j * 512:(j + 1) * 512],
                            kT[hsl, b * TKV + kc * 128:b * TKV + (kc + 1) * 128],
                            qT[hsl, qsl], start=True, stop=True)
                    pt = ptp.tile([128, 1024], F16, tag="pt", name="pt")
                    nc.scalar.activation(pt[:], st[:], AF.Exp)
                    if use_mask:
                        nc.vector.tensor_mul(pt[:], pt[:], mts[half][:])
                    pts.append(pt)
                return pts

            def pv_norm(b, qb, h, pts):
                """PV with q on partitions, denominator col, normalize+evac."""
                ov = pvp.tile([128, 4 * (DH + 1)], F32, tag="pv", name="ov")
                for qt in range(4):
                    osl = slice(qt * (DH + 1), (qt + 1) * (DH + 1))
                    for kc in range(KVC):
                        pcol = (kc % 2) * 512 + qt * 128
                        nc.tensor.matmul(
                            ov[:, osl],
                            pts[kc // 2][:, pcol:pcol + 128],
                            vn3[:, b * KVC + kc, h * (DH + 1):
                                (h + 1) * (DH + 1)],
                            start=(kc == 0), stop=(kc == KVC - 1))
                s_off = 0 if h == 0 else DH       # ones col position per head
                c_off = 1 if h == 0 else 0
                for qt in range(4):
                    base = qt * (DH + 1)
                    rec = smlp.tile([128, 1], F32, tag="rec", name="rec")
                    nc.vector.reciprocal(
                        rec[:], ov[:, base + s_off:base + s_off + 1])
                    otq = otqp.tile([128, DH], F16, tag="otq", name="otq")
                    nc.vector.tensor_scalar(
                        otq[:], ov[:, base + c_off:base + c_off + DH],
                        rec[:], None, ALU.mult)
                    otq_tiles[(h, qt)] = otq

            def transposes(b, qb):
                """[q, chan] -> ot_t[chan, tok] via identity matmuls."""
                for qt in range(4):
                    tr = pvp.tile([128, 4 * (DH + 1)], F32, tag="pv",
                                  name="tr")
                    for h in range(HPC):
                        nc.tensor.matmul(tr[h * DH:(h + 1) * DH, 0:128],
                                         otq_tiles[(h, qt)][:],
                                         idpm[:, 0:128],
                                         start=True, stop=True)
                    col = b * TQ + qb * 512 + qt * 128
                    nc.vector.tensor_copy(ot_t[:, col:col + 128],
                                          tr[:, 0:128])

            osb_tiles = {}

            def outproj_quarter(b, jc, tb4, evac_act=False):
                """one [128,512] token-quarter of output row-block jc."""
                key = (b, jc)
                if key not in osb_tiles:
                    osb_tiles[key] = osb.tile([128, 2048], F16, tag="o_sb",
                                              name="o_sb")
                o_big = osb_tiles[key]
                o_ps = pjp.tile([128, 512], F32, tag="pj", name="o_ps")
                col = b * TQ + tb4 * 512
                nc.tensor.matmul(o_ps[:], woT[:, jc * 128:(jc + 1) * 128],
                                 ot_t[:, col:col + 512], start=True, stop=True)
                c0 = tb4 * 512
                if evac_act:     # tail quarters: Act is idle after last exp
                    nc.scalar.activation(o_big[:, c0:c0 + 512], o_ps[:],
                                         AF.Identity)
                else:
                    nc.vector.tensor_copy(o_big[:, c0:c0 + 512], o_ps[:])
                nc.gpsimd.dma_start(
                    outT[jc * 128:(jc + 1) * 128,
                         b * TQ + c0:b * TQ + c0 + 512],
                    o_big[:, c0:c0 + 512])

            mask_tiles = {}

            def load_mask(b, qb):
                if not use_mask or (b, qb) in mask_tiles:
                    return
                mts = []
                for half in range(KVC // 2):
                    mt = mkp.tile([128, 1024], F16, tag="mk", name="mt")
                    for j in range(2):
                        kc = half * 2 + j
                        nc.sync.dma_start(
                            mt[:, j * 512:(j + 1) * 512],
                            mask_d[kc * 128:(kc + 1) * 128, b,
                                   qb * 512:(qb + 1) * 512])
                    mts.append(mt)
                mask_tiles[(b, qb)] = mts

            # -------- schedule ----------------------------------------------
            # kv-side of b0 projects first and the first combo's QK halves
            # interleave with the remaining kv blocks, so exp starts as soon
            # as kv blocks 0-1 + q block 0 are rotated. The remaining q/kv
            # blocks and the output projection are interleaved into the
            # attention pipeline as PE fillers inside Act-bound exp windows.
            # One-combo-deep software pipeline: QK+exp of combo i issues
            # before PV of combo i-1. DMAs are emitted just-in-time so SP's
            # in-order queue feeds the startup critical path first.
            # critical path to the first exp: kvk0, q0, kvk1 projections +
            # rotaries (splices on Act's queue), QK halves as kv blocks land;
            # v-projections are deferred into the early exp windows.
            kv_part(0, pre=pre01kv[:, :, 0:512])
            kv_part(1, pre=pre01kv[:, :, 512:1024])
            q_part(0, pre=pre0q)
            nc.sync.dma_start(tabk[:, 2048:4096], tabk_d[:, 2048:4096])
            load_mask(0, 0)
            kv_part(2, pre=pre2kv)
            pts0 = qk_exp(0, 0, 0, halves=[0, 1, 2, 3])
            qk_exp(0, 0, 0, halves=[4, 5], pts=pts0)
            kv_part(3)
            nc.sync.dma_start(tabk[:, 4096:8192], tabk_d[:, 4096:8192])
            qk_exp(0, 0, 0, halves=[6, 7], pts=pts0)
            nc.sync.dma_start(tabq[:, 4096:8192], tabq_d[:, 4096:8192])
            nc.sync.dma_start(woT[:], woT_d[:])

            combos = [(b, qb, h) for b in range(B) for qb in range(QB)
                      for h in range(HPC)]
            fillers = {
                (0, 0, 1): [("q", 1)],
                (0, 1, 0): [("kv", 4)], (0, 1, 1): [("q", 2)],
                (0, 2, 0): [("kv", 5)], (0, 2, 1): [("q", 3), ("q", 4)],
                (0, 3, 0): [("kv", 6)], (0, 3, 1): [("kv", 7)],
                (1, 0, 0): [("q", 5)],
                (1, 1, 0): [("q", 6)],
                (1, 2, 0): [("q", 7)],
            }
            # outproj quarters: (0,*) available from combo index 9 (after
            # transposes(0,3) at index 8); (1,*,tb4) needs transposes(1,tb4),
            # emitted during combo index 10+2*tb4.
            opq = {9: [(0, 0), (0, 1)], 10: [(0, 2), (0, 3)],
                   11: [(0, 4), (0, 5)], 12: [(0, 6), (0, 7)]}
            for i, jcs in opq.items():
                fillers.setdefault(combos[i], []).extend(
                    ("op", b, jc, tb4) for b, jc in jcs for tb4 in range(4))
            h0slots = {13: [0, 1, 2], 14: [3, 4, 5], 15: [6, 7]}
            for i, jcs in h0slots.items():
                fillers.setdefault(combos[i], []).extend(
                    ("op", 1, jc, tb4) for jc in jcs for tb4 in (0, 1))
            # t2 quarters fit in the final exp window (transposes(1,2) are
            # emitted during combo 14)
            fillers.setdefault(combos[15], []).extend(
                ("op", 1, jc, 2) for jc in range(8))
            prev = (0, 0, 0)
            pts_prev = pts0
            for c in combos[1:]:
                load_mask(c[0], c[1])
                pts = qk_exp(*c)
                for f in fillers.get(c, []):
                    if f[0] == "q":
                        q_part(f[1])
                    elif f[0] == "kv":
                        kv_part(f[1])
                    else:
                        outproj_quarter(f[1], f[2], f[3])
                if prev is not None:
                    pv_norm(*prev, pts_prev)
                    if prev[2] == HPC - 1:
                        transposes(prev[0], prev[1])
                prev, pts_prev = c, pts
            pv_norm(*prev, pts_prev)
            transposes(prev[0], prev[1])
            for jc in range(8):
                outproj_quarter(1, jc, 3, evac_act=True)

    nc.compile()
    return nc


# ---------------------------------------------------------------- pjrt runner
def _make_runner(nc, n_cores=NCORES):
    import jax
    from jax.sharding import Mesh, PartitionSpec
    from jax.experimental.shard_map import shard_map
    from concourse.bass2jax import (_bass_exec_p, install_neuronx_cc_hook,
                                    partition_id_tensor)

    install_neuronx_cc_hook()
    partition_name = (nc.partition_id_tensor.name
                      if nc.partition_id_tensor else None)
    in_names, out_names, out_avals, zero_shapes = [], [], [], []
    for alloc in nc.m.functions[0].allocations:
        if not isinstance(alloc, mybir.MemoryLocationSet):
            continue
        name = alloc.memorylocations[0].name
        if alloc.kind == "ExternalInput":
            if name != partition_name:
                in_names.append(name)
        elif alloc.kind == "ExternalOutput":
            shape = tuple(alloc.tensor_shape)
            dtype = mybir.dt.np(alloc.dtype)
            out_names.append(name)
            out_avals.append(jax.core.ShapedArray(shape, dtype))
            zero_shapes.append((shape, dtype))
    n_params = len(in_names)
    n_outs = len(out_avals)
    all_in_names = list(in_names) + list(out_names)
    if partition_name is not None:
        all_in_names.append(partition_name)

    def _body(*args):
        operands = list(args)
        if partition_name is not None:
            operands.append(partition_id_tensor())
        return tuple(_bass_exec_p.bind(
            *operands, out_avals=tuple(out_avals), in_names=tuple(all_in_names),
            out_names=tuple(out_names), lowering_input_output_aliases=(),
            sim_require_finite=True, sim_require_nnan=True, nc=nc))

    devices = jax.devices()[:n_cores]
    mesh = Mesh(np.asarray(devices), ("core",))
    in_specs = (PartitionSpec("core"),) * (n_params + n_outs)
    out_specs = (PartitionSpec("core"),) * len(out_names)
    donate = tuple(range(n_params, n_params + n_outs))
    sharded = jax.jit(
        shard_map(_body, mesh=mesh, in_specs=in_specs, out_specs=out_specs,
                  check_rep=False),
        donate_argnums=donate, keep_unused=True)

    def run(in_maps, time_iters=0):
        per_core = [[np.asarray(m[name]) for name in in_names]
                    for m in in_maps]
        concat_in = [np.concatenate([per_core[c][i] for c in range(n_cores)],
                                    axis=0) for i in range(n_params)]

        def zeros():
            return [np.zeros((n_cores * s[0], *s[1:]), d)
                    for s, d in zero_shapes]

        import jax
        out_arrs = sharded(*concat_in, *zeros())
        jax.block_until_ready(out_arrs)
        times = []
        for _ in range(time_iters):
            t0 = time.perf_counter()
            o = sharded(*concat_in, *zeros())
            jax.block_until_ready(o)
            times.append(time.perf_counter() - t0)
            out_arrs = o
        results = [
            {name: np.asarray(out_arrs[i]).reshape(n_cores,
                                                   *out_avals[i].shape)[c]
             for i, name in enumerate(out_names)}
            for c in range(n_cores)]
        return results, times

    return run


# ---------------------------------------------------------------- host shard
def _tables(positions, core, npos_dtype=np.float64):
    """cos/sin [128, T] fp16 tables; sign of the splice folded into sin."""
    pos = np.asarray(positions, np.float64).T.reshape(T)   # b-major tokens
    nb = NROT // 2
    freq = MAX_WL ** (2.0 / NROT * np.linspace(0.0, float(nb), nb))
    inv = 1.0 / freq                                        # [16]
    cos = np.ones((128, T), np.float64)
    sin = np.zeros((128, T), np.float64)
    for hl in range(HPC):
        hglob = core * HPC + hl
        if hglob >= NHEADS_ROT:
            continue
        for cc in range(NROT):
            ang = pos * inv[cc // 2]
            r = hl * DH + cc
            cos[r] = np.cos(ang)
            sgn = -1.0 if cc % 2 == 0 else 1.0
            sin[r] = sgn * np.sin(ang)
    return cos.astype(NPF16), sin.astype(NPF16)


def _pack_tab(cos, sin):
    """[cos_blk | sin_blk] interleaved per 512-token block."""
    chunks = []
    for tb in range(TB):
        chunks.append(cos[:, tb * 512:(tb + 1) * 512])
        chunks.append(sin[:, tb * 512:(tb + 1) * 512])
    return np.ascontiguousarray(np.concatenate(chunks, axis=1))


def make_in_maps(inputs_q, inputs_kv, mask, q_positions, kv_positions,
                 Wq, bq, Wk, bk, Wv, bv, Wo, bo, use_mask):
    f32 = np.float32
    xqT = np.ascontiguousarray(
        np.asarray(inputs_q, f32).transpose(2, 1, 0).reshape(D, T)).astype(NPF16)
    xkvT = np.ascontiguousarray(
        np.asarray(inputs_kv, f32).transpose(2, 1, 0).reshape(D, T)).astype(NPF16)
    scale = f32(1.0 / np.sqrt(DH))
    Wq, Wk, Wv, Wo = (np.asarray(a, f32) for a in (Wq, Wk, Wv, Wo))
    bq, bk, bv, bo = (np.asarray(a, f32) for a in (bq, bk, bv, bo))
    iden = np.eye(128, dtype=NPF16)
    perm = np.zeros((128, 128), NPF16)
    perm[np.arange(128), np.arange(128) ^ 1] = 1.0
    idpm = np.concatenate([iden, perm], axis=1)
    if use_mask:
        maskT = np.ascontiguousarray((np.asarray(mask) > 0).astype(NPF16))

    in_maps = []
    for c in range(NCORES):
        sl = slice(c * MPC, (c + 1) * MPC)
        cq, sq = _tables(q_positions, c)
        ck, sk = _tables(kv_positions, c)
        w3 = np.concatenate(
            [Wk[sl, :].T, Wv[sl, :].T, (scale * Wq[sl, :]).T],
            axis=1)
        bqk = np.stack([bk[sl], scale * bq[sl]], axis=1)
        m = {
            "xqT": xqT, "xkvT": xkvT,
            "w3": np.ascontiguousarray(w3).astype(NPF16),
            "bqk": np.ascontiguousarray(bqk, np.float32),
            "woT": np.ascontiguousarray(Wo[:, sl].T).astype(NPF16),
            "tabq": _pack_tab(cq, sq),
            "tabk": _pack_tab(ck, sk),
            "idpm": idpm,
        }
        if use_mask:
            m["maskT"] = maskT
        in_maps.append(m)
    return in_maps


_CACHE = {}


def _get(use_mask):
    if use_mask not in _CACHE:
        nc = build_kernel(use_mask)
        _CACHE[use_mask] = (nc, _make_runner(nc))
    return _CACHE[use_mask]


def kernel(inputs_q, inputs_kv, mask, q_positions, kv_positions,
           Wq, bq, Wk, bk, Wv, bv, Wo, bo, _time_iters=0):
    use_mask = not bool(np.all(np.asarray(mask) > 0))
    nc, run = _get(use_mask)
    in_maps = make_in_maps(inputs_q, inputs_kv, mask, q_positions,
                           kv_positions, Wq, bq, Wk, bk, Wv, bv, Wo, bo,
                           use_mask)
    results, times = run(in_maps, time_iters=_time_iters)
    acc = np.zeros((D, T), np.float64)
    for c in range(NCORES):
        acc += results[c]["outT"].astype(np.float64)
    bo_full = (np.asarray(Wo, np.float64) @ np.asarray(bv, np.float64)
               + np.asarray(bo, np.float64))
    acc += bo_full[:, None]
    out = acc.astype(np.float32).reshape(D, B, TQ).transpose(2, 1, 0)
    out = np.ascontiguousarray(out)
    if _time_iters:
        kernel._last_times = times
    return out
